# revision 1
# baseline (speedup 1.0000x reference)
"""Trainium2 Bass kernel for BaseAttnPredictNet (pre-LN multi-head attention
with zero-attn slot, gated output combination, residual).

Sharding: data-parallel over (batch, query-rows). 8 cores, each takes 512
query rows (cores 0-3 -> batch 0, cores 4-7 -> batch 1). Each core
redundantly computes the K/V projections for its batch; no collectives.

On-device layout is "transposed world": activations as [feature, row]
(feature on partitions) so every matmul is out = W.T @ xT; the only
activation transposes are the LN outputs (PE transposes, bf16).

Softmax: no max-subtraction (scores are ~N(0,1), bounded); key mask folded
into the exp as a per-partition bias (0 or -1e30); denominator via an
appended ones-column in the PV matmul; LN gamma folded into the weights,
LN beta (if nonzero) via projected bias terms.
"""

import numpy as np

import concourse.bass as bass
import concourse.bacc as bacc
import concourse.mybir as mybir
import concourse.tile as tile
from concourse.bass_utils import run_bass_kernel_spmd
from concourse.masks import make_identity

# problem shapes (hardcoded per contract)
B, Q, KLEN, D = 2, 2048, 2048, 512
H, DH = 8, 64
P = 128
KP = 2176  # padded key length: 2048 real + 1 zero-attn slot + 127 masked pad
NJ = KP // P  # 17 key blocks
QS = 512  # query rows per core
NI = QS // P  # 4 row blocks
ND = D // P  # 4 feature blocks
NG = 2 * D // P  # 8 gate-contraction blocks
NCORES = 8
SCALE = 0.125
LN_EPS = 1e-5
NEGBIG = -1e30

F32 = mybir.dt.float32
BF16 = mybir.dt.bfloat16
AF = mybir.ActivationFunctionType
OP = mybir.AluOpType


def _build(use_gamma: bool, use_beta: bool, reps: int = 1) -> bass.Bass:
    """reps>1 unrolls the whole body N times (same I/O) for delta-timing."""
    nc = bacc.Bacc("TRN2", target_bir_lowering=False, debug=False)

    din = {}
    for name, shape in (
        ("q", [QS, D]),
        ("k", [KP, D]),
        ("v", [KP, D]),
        ("wq", [D, D]),
        ("wk", [D, D]),
        ("wv", [D, D]),
        ("wo", [D, D]),
        ("gw", [2 * D, D]),
        ("gb", [P, ND]),
        ("kmb", [P, NJ]),
        ("qm", [1, QS]),
    ):
        din[name] = nc.dram_tensor(name, shape, F32, kind="ExternalInput")
    if use_gamma:
        for name in ("qg", "kg", "vg"):
            din[name] = nc.dram_tensor(name, [P, ND], F32, kind="ExternalInput")
    if use_beta:
        for name in ("qb", "kb", "vb"):
            din[name] = nc.dram_tensor(name, [P, ND], F32, kind="ExternalInput")
    out_d = nc.dram_tensor("out", [QS, D], F32, kind="ExternalOutput")

    with tile.TileContext(nc) as tc:
        for i in range(reps):
            # chain reps through the output tensor so DCE keeps every copy
            q_src = din["q"] if i == 0 else out_d
            _body(nc, tc, din, out_d, use_gamma, use_beta, q_src=q_src)
    nc.compile()
    return nc


def _body(nc, tc, din, out_d, use_gamma, use_beta, q_src=None):
    if q_src is None:
        q_src = din["q"]
    from contextlib import ExitStack

    ctx = ExitStack()
    with ctx:
        persist = ctx.enter_context(tc.tile_pool(name="persist", bufs=1))
        stats = ctx.enter_context(tc.tile_pool(name="stats", bufs=6))
        # PSUM pools: 2 (pacc) + 4 (pS 2-bank x2) + 2 (pav) = 8 banks
        pacc = ctx.enter_context(tc.tile_pool(name="pacc", bufs=3, space="PSUM"))
        pS = ctx.enter_context(tc.tile_pool(name="pS", bufs=2, space="PSUM"))
        pav = ctx.enter_context(tc.tile_pool(name="pav", bufs=1, space="PSUM"))

        ident_bf = persist.tile([P, P], BF16)
        make_identity(nc, ident_bf)
        ident_f32 = persist.tile([P, P], F32)
        make_identity(nc, ident_f32)
        eps_t = persist.tile([P, 1], F32)
        nc.vector.memset(eps_t, LN_EPS)
        km01 = persist.tile([P, NJ], F32)
        nc.sync.dma_start(out=km01, in_=din["kmb"][:, :])
        qm_bc = persist.tile([P, QS], F32)
        _qm_ap = din["qm"][:, :]
        nc.sync.dma_start(
            out=qm_bc,
            in_=bass.AP(tensor=_qm_ap.tensor, offset=_qm_ap.offset, ap=[[0, P], [1, QS]]),
        )
        gb = persist.tile([P, ND], F32)
        nc.sync.dma_start(out=gb, in_=din["gb"][:, :])

        gam = {}
        if use_gamma:
            for nm in ("qg", "kg", "vg"):
                g = persist.tile([P, ND], F32, name=nm)
                nc.sync.dma_start(out=g, in_=din[nm][:, :])
                gam[nm] = g
        bet = {}
        if use_beta:
            for nm in ("qb", "kb", "vb"):
                bt = persist.tile([P, ND], F32, name=nm)
                nc.sync.dma_start(out=bt, in_=din[nm][:, :])
                bet[nm] = bt

        # xstage first: DMA-destination memory must never sit on recycled
        # pool space (walrus caps DMA waits at 2; recycled regions accumulate
        # cross-lane deps that Tile will not prune transitively)
        # bufs matched to the 8 SWDGE sem lanes: same-slot DMA predecessors
        # then share one lane, keeping every DMA at <=2 encoded waits.
        xstage = tc.alloc_tile_pool(name="xstage", bufs=8)
        # ---- weights: DMA per matrix-row-block into staging, cast to bf16 ----
        wstage = tc.alloc_tile_pool(name="wstage", bufs=8)
        w_bf = {}
        bxT = {}  # per-matrix (beta @ W)^T as [128, ND] (partition-major over n)
        for wname, gname, bname in (
            ("wq", "qg", "qb"),
            ("wk", "kg", "kb"),
            ("wv", "vg", "vb"),
            ("wo", None, None),
            ("gw", None, None),
        ):
            nblk = NG if wname == "gw" else ND
            wb = persist.tile([P, nblk, D], BF16, name=f"{wname}_bf")
            for b in range(nblk):
                wf = wstage.tile([P, D], F32, name="wstage_t")
                nc.sync.dma_start(out=wf, in_=din[wname][b * P : (b + 1) * P, :])
                if use_gamma and gname is not None:
                    nc.vector.tensor_scalar(
                        out=wb[:, b, :],
                        in0=wf,
                        scalar1=gam[gname][:, b : b + 1],
                        scalar2=None,
                        op0=OP.mult,
                    )
                else:
                    nc.vector.tensor_copy(wb[:, b, :], wf)
            w_bf[wname] = wb
            if use_beta and bname is not None:
                # bxT [n,1] per n-blk: lhsT=W [d, n-blk], rhs=beta [d,1]
                bx = persist.tile([P, ND], F32, name=f"bx_{wname}")
                betb = persist.tile([P, ND], BF16, name=f"betb_{wname}")
                nc.vector.tensor_copy(betb, bet[bname])
                for a in range(ND):
                    pb = pacc.tile([P, 1], F32, name="pacc_t")
                    for b in range(nblk):
                        nc.tensor.matmul(
                            pb,
                            wb[:, b, a * P : (a + 1) * P],
                            betb[:, b : b + 1],
                            start=(b == 0),
                            stop=(b == nblk - 1),
                        )
                    nc.vector.tensor_copy(bx[:, a : a + 1], pb)
                bxT[wname] = bx
        wstage.release()

        # ---- persistent activation tensors ----
        qT_f = persist.tile([P, ND, QS], F32)
        qhT = [persist.tile([P, QS], BF16, name=f"qhT{a}") for a in range(ND)]
        khT = [persist.tile([P, KP], BF16, name=f"khT{a}") for a in range(ND)]
        vh_aug = [
            persist.tile([P, H, DH + 1], BF16, name=f"vha{c}") for c in range(NJ)
        ]
        av_nat = persist.tile([P, NI, D], BF16)
        avT = persist.tile([P, ND, QS], BF16)
        poT_f = persist.tile([P, ND, QS], F32)
        gT = persist.tile([P, ND, QS], F32)
        outT = persist.tile([P, ND, QS], F32)

        def ln_scales_chunk(xf, cw):
            """Per-row LN (bias, scale) for up to 2 blocks of a chunk tile.
            Returns (nm2 [P, cw], rstd [P, cw]): batches the ACT sqrt."""
            mv = stats.tile([P, 2, 2], F32, name="bnagg")
            for cc in range(cw):
                st = stats.tile([P, 6], F32, name="bnst")
                nc.vector.bn_stats(out=st, in_=xf[:, cc, :])
                nc.vector.bn_aggr(out=mv[:, cc, :], in_=st)
            std = stats.tile([P, 2], F32, name="std")
            nc.scalar.activation(
                out=std[:, :cw], in_=mv[:, 0:cw, 1], func=AF.Sqrt, bias=eps_t
            )
            rstd = stats.tile([P, 2], F32, name="rstd")
            nc.vector.reciprocal(rstd[:, :cw], std[:, :cw])
            nm2 = stats.tile([P, 2], F32, name="nm2")
            nc.vector.tensor_tensor(
                out=nm2[:, :cw], in0=mv[:, 0:cw, 0], in1=rstd[:, :cw], op=OP.mult
            )
            nc.vector.tensor_scalar_mul(nm2[:, :cw], nm2[:, :cw], -1.0)
            return nm2, rstd

        def load_chunk(src_dram, r0, cw):
            """DMA rows [r0*P, (r0+cw)*P) as one transfer -> [P, cw, D]."""
            xf = xstage.tile([P, 2, D], F32, name="xstage_t")
            base = q_src if src_dram == "q" else din[src_dram]
            src = base[r0 * P : (r0 + cw) * P, :].rearrange(
                "(c p) d -> p c d", p=P
            )
            nc.sync.dma_start(out=xf[:, :cw, :], in_=src)
            return xf

        def ln_transpose(src_dram, nrows, xnT_dest):
            """Stream rows: LN (DVE) -> bf16 -> PE-transpose into
            xnT_dest [P, ND, nrows]."""
            nblk = nrows // P
            for c0 in range(0, nblk, 2):
                cw = min(2, nblk - c0)
                xf = load_chunk(src_dram, c0, cw)
                xn_chunk = []
                nm2, rstd = ln_scales_chunk(xf, cw)
                for cc in range(cw):
                    xn = stats.tile([P, D], BF16, name="xnorm")
                    nc.vector.tensor_scalar(
                        out=xn,
                        in0=xf[:, cc, :],
                        scalar1=nm2[:, cc : cc + 1],
                        scalar2=rstd[:, cc : cc + 1],
                        op0=OP.add,
                        op1=OP.mult,
                    )
                    xn_chunk.append(xn)
                for b in range(ND):
                    pt = pacc.tile([P, 2 * P], BF16, name="pacc_t")
                    for cc in range(cw):
                        nc.tensor.transpose(
                            pt[:, cc * P : (cc + 1) * P],
                            xn_chunk[cc][:, b * P : (b + 1) * P],
                            ident_bf,
                        )
                    nc.vector.tensor_copy(
                        xnT_dest[:, b, c0 * P : (c0 + cw) * P], pt[:, : cw * P]
                    )

        pa_qk = tc.alloc_tile_pool(name="pa_qk", bufs=1)
        qnT = pa_qk.tile([P, ND, QS], BF16)
        knT = pa_qk.tile([P, ND, KP], BF16)

        # ---- q: raw transpose (f32 + bf16 casts) and LN transpose ----
        qf0 = load_chunk("q", 0, 2)
        qf1 = load_chunk("q", 2, 2)
        qparts = [qf0[:, 0, :], qf0[:, 1, :], qf1[:, 0, :], qf1[:, 1, :]]
        qcopy = xstage.tile([P, ND, D], F32, name="qcopy", bufs=1)
        for cc in range(NI):
            nc.gpsimd.tensor_copy(qcopy[:, cc, :], qparts[cc])
        for b in range(ND):
            pt = pacc.tile([P, 4 * P], F32, name="pacc_t")
            for cc in range(NI):
                nc.tensor.transpose(
                    pt[:, cc * P : (cc + 1) * P],
                    qcopy[:, cc, b * P : (b + 1) * P],
                    ident_f32,
                )
            nc.vector.tensor_copy(qT_f[:, b, :], pt)
        qn_chunk = []
        for qf in (qf0, qf1):
            nm2, rstd = ln_scales_chunk(qf, 2)
            for cc in range(2):
                xn = stats.tile([P, D], BF16, name="xnorm")
                nc.vector.tensor_scalar(
                    out=xn,
                    in0=qf[:, cc, :],
                    scalar1=nm2[:, cc : cc + 1],
                    scalar2=rstd[:, cc : cc + 1],
                    op0=OP.add,
                    op1=OP.mult,
                )
                qn_chunk.append(xn)
        for b in range(ND):
            pt = pacc.tile([P, 4 * P], BF16, name="pacc_t")
            for cc in range(NI):
                nc.tensor.transpose(
                    pt[:, cc * P : (cc + 1) * P],
                    qn_chunk[cc][:, b * P : (b + 1) * P],
                    ident_bf,
                )
            nc.vector.tensor_copy(qnT[:, b, :], pt)

        # ---- k: LN + transpose ----
        ln_transpose("k", KP, knT)

        # ---- q/k projections ----
        # qhT [n, i] = Wq'.T @ qnT
        for a in range(ND):
            pp = pacc.tile([P, QS], F32, name="pacc_t")
            for b in range(ND):
                nc.tensor.matmul(
                    pp,
                    w_bf["wq"][:, b, a * P : (a + 1) * P],
                    qnT[:, b, :],
                    start=(b == 0),
                    stop=(b == ND - 1),
                )
            if use_beta:
                nc.vector.tensor_scalar(
                    out=qhT[a],
                    in0=pp,
                    scalar1=bxT["wq"][:, a : a + 1],
                    scalar2=None,
                    op0=OP.add,
                )
            else:
                nc.vector.tensor_copy(qhT[a], pp)
        # khT [n, j] = Wk'.T @ knT   (j in chunks of 512)
        for a in range(ND):
            for j0 in range(0, KP, 512):
                jw = min(512, KP - j0)
                pp = pacc.tile([P, QS], F32, name="pacc_t")
                for b in range(ND):
                    nc.tensor.matmul(
                        pp[:, :jw],
                        w_bf["wk"][:, b, a * P : (a + 1) * P],
                        knT[:, b, j0 : j0 + jw],
                        start=(b == 0),
                        stop=(b == ND - 1),
                    )
                if use_beta:
                    nc.vector.tensor_scalar(
                        out=khT[a][:, j0 : j0 + jw],
                        in0=pp[:, :jw],
                        scalar1=bxT["wk"][:, a : a + 1],
                        scalar2=None,
                        op0=OP.add,
                    )
                else:
                    nc.scalar.copy(khT[a][:, j0 : j0 + jw], pp[:, :jw])
        pa_qk.release()

        # ---- v: LN + transpose, then vh ----
        pa_v = tc.alloc_tile_pool(name="pa_v", bufs=1)
        vnT = pa_v.tile([P, ND, KP], BF16)
        ln_transpose("v", KP, vnT)
        # vh natural [j, n] = vnT.T @ Wv', into vh_aug (65-strided heads)
        for c in range(NJ):
            pp = pacc.tile([P, QS], F32, name="pacc_t")
            for b in range(ND):
                nc.tensor.matmul(
                    pp,
                    vnT[:, b, c * P : (c + 1) * P],
                    w_bf["wv"][:, b, :],
                    start=(b == 0),
                    stop=(b == ND - 1),
                )
            pp3 = pp.rearrange("p (h e) -> p h e", h=H)
            nc.scalar.activation(
                out=vh_aug[c][:, :, 0:DH],
                in_=pp3,
                func=AF.Copy,
                scale=km01[:, c : c + 1],
            )
            nc.vector.tensor_copy(
                vh_aug[c][:, :, DH : DH + 1],
                km01[:, c : c + 1].unsqueeze(1).broadcast_to((P, H, 1)),
            )
        pa_v.release()
        xstage.release()

        # ---- attention, head by head ----
        pb_attn = ctx.enter_context(tc.tile_pool(name="pb_attn", bufs=2))
        for h in range(H):
            nb = h // 2
            r0 = (h % 2) * DH
            expS = []
            for c0 in range(0, NJ, 2):
                cw = min(2, NJ - c0)
                ps2 = pS.tile([P, 2, QS], F32, name="pS2")
                for i in range(cw):
                    c = c0 + i
                    nc.tensor.matmul(
                        ps2[:, i, :],
                        khT[nb][r0 : r0 + DH, c * P : (c + 1) * P],
                        qhT[nb][r0 : r0 + DH, :],
                        start=True,
                        stop=True,
                    )
                e2 = pb_attn.tile([P, 2, QS], BF16, name="expS", bufs=2 * (NJ // 2 + 1))
                nc.scalar.activation(
                    out=e2[:, :cw, :], in_=ps2[:, :cw, :], func=AF.Exp, scale=SCALE
                )
                for i in range(cw):
                    expS.append(e2[:, i, :])
            for a in range(NI):
                pv = pav.tile([P, DH + 1], F32, name="pav_t")
                for c in range(NJ):
                    nc.tensor.matmul(
                        pv,
                        expS[c][:, a * P : (a + 1) * P],
                        vh_aug[c][:, h, :],
                        start=(c == 0),
                        stop=(c == NJ - 1),
                    )
                rden = stats.tile([P, 1], F32, name="rden")
                nc.vector.reciprocal(rden, pv[:, DH : DH + 1])
                nc.vector.tensor_scalar(
                    out=av_nat[:, a, h * DH : (h + 1) * DH],
                    in0=pv[:, 0:DH],
                    scalar1=rden,
                    scalar2=None,
                    op0=OP.mult,
                )

        # ---- avT (with query-mask fold; beta_v enters here since
        # sum(attn)=1 makes +bv commute with the softmax average) ----
        for b in range(ND):
            pt = pacc.tile([P, 4 * P], BF16, name="pacc_t")
            for a in range(NI):
                nc.tensor.transpose(
                    pt[:, a * P : (a + 1) * P],
                    av_nat[:, a, b * P : (b + 1) * P],
                    ident_bf,
                )
            if use_beta:
                tbv = pb_attn.tile([P, QS], BF16, name="tbv")
                nc.vector.tensor_scalar(
                    out=tbv, in0=pt, scalar1=bxT["wv"][:, b : b + 1],
                    scalar2=None, op0=OP.add,
                )
                nc.vector.tensor_tensor(out=avT[:, b, :], in0=tbv, in1=qm_bc, op=OP.mult)
            else:
                nc.vector.tensor_tensor(out=avT[:, b, :], in0=pt, in1=qm_bc, op=OP.mult)

        # ---- output projection poT = Wo.T @ avT ----
        for a in range(ND):
            pp = pacc.tile([P, QS], F32, name="pacc_t")
            for b in range(ND):
                nc.tensor.matmul(
                    pp,
                    w_bf["wo"][:, b, a * P : (a + 1) * P],
                    avT[:, b, :],
                    start=(b == 0),
                    stop=(b == ND - 1),
                )
            nc.vector.tensor_copy(poT_f[:, a, :], pp)

        # ---- gate gT = sigmoid(gw.T @ [qT; poT] + gb) ----
        gate_rhs = []
        for b in range(NG):
            src = qT_f[:, b, :] if b < ND else poT_f[:, b - ND, :]
            cb = pb_attn.tile([P, QS], BF16, name="gatecast", bufs=8)
            nc.gpsimd.tensor_copy(cb, src)
            gate_rhs.append(cb)
        for a in range(ND):
            pp = pacc.tile([P, QS], F32, name="pacc_t")
            for b in range(NG):
                rhs = gate_rhs[b]
                nc.tensor.matmul(
                    pp,
                    w_bf["gw"][:, b, a * P : (a + 1) * P],
                    rhs,
                    start=(b == 0),
                    stop=(b == NG - 1),
                )
            nc.scalar.activation(
                out=gT[:, a, :], in_=pp, func=AF.Sigmoid, bias=gb[:, a : a + 1]
            )

        # ---- final combine: out = q + po + g*(q - po) ----
        for a in range(ND):
            s = pb_attn.tile([P, QS], F32, name="fin_t", bufs=6)
            nc.vector.tensor_tensor(
                out=s, in0=qT_f[:, a, :], in1=poT_f[:, a, :], op=OP.subtract
            )
            m = pb_attn.tile([P, QS], F32, name="fin_t", bufs=6)
            nc.vector.tensor_tensor(out=m, in0=gT[:, a, :], in1=s, op=OP.mult)
            r = pb_attn.tile([P, QS], F32, name="fin_t", bufs=6)
            nc.gpsimd.tensor_tensor(
                out=r, in0=qT_f[:, a, :], in1=poT_f[:, a, :], op=OP.add
            )
            nc.vector.tensor_tensor(out=outT[:, a, :], in0=m, in1=r, op=OP.add)

        # ---- transpose back + one combined store ----
        out_nat = persist.tile([P, NI, D], F32, name="outn")
        for a in range(NI):
            pt = pacc.tile([P, 4 * P], F32, name="pacc_t")
            for b in range(ND):
                nc.tensor.transpose(
                    pt[:, b * P : (b + 1) * P],
                    outT[:, b, a * P : (a + 1) * P],
                    ident_f32,
                )
            nc.vector.tensor_copy(out_nat[:, a, :], pt)
        dst = out_d[:, :].rearrange("(a p) d -> p a d", p=P)
        nc.sync.dma_start(out=dst, in_=out_nat)


_CACHE: dict = {}


def make_in_maps(inputs):
    """Shard full inputs into per-core input maps; returns (in_maps, flags)."""
    q = np.asarray(inputs["query"], np.float32)
    k = np.asarray(inputs["key"], np.float32)
    v = np.asarray(inputs["value"], np.float32)
    wq = np.asarray(inputs["weight_q"], np.float32)
    wk = np.asarray(inputs["weight_k"], np.float32)
    wv = np.asarray(inputs["weight_v"], np.float32)
    wo = np.asarray(inputs["weight_o"], np.float32)
    gw = np.asarray(inputs["g_w"], np.float32)
    gb = np.asarray(inputs["g_b"], np.float32)
    qmask = np.asarray(inputs["query_mask"])
    kmask = np.asarray(inputs["key_mask"])
    gams = [
        np.asarray(inputs[n], np.float32) for n in ("q_gamma", "k_gamma", "v_gamma")
    ]
    bets = [np.asarray(inputs[n], np.float32) for n in ("q_beta", "k_beta", "v_beta")]

    use_gamma = any(not np.allclose(g, 1.0) for g in gams)
    use_beta = any(np.any(bt != 0.0) for bt in bets)

    def colmajor(vec):  # [D] -> [128, ND] partition-major
        return np.ascontiguousarray(vec.reshape(-1, P).T)

    # padded K/V + per-key exp bias (0 = attend, -1e30 = masked)
    kpad = np.zeros((B, KP, D), np.float32)
    vpad = np.zeros((B, KP, D), np.float32)
    kpad[:, :KLEN] = k
    vpad[:, :KLEN] = v
    kmb = np.zeros((B, KP), np.float32)
    kmb[:, :KLEN] = (kmask != 0).astype(np.float32)
    kmb[:, KLEN] = 1.0  # zero-attn slot always attendable

    per_batch = NCORES // B
    in_maps = []
    for c in range(NCORES):
        b, r = c // per_batch, c % per_batch
        m = {
            "q": np.ascontiguousarray(q[b, r * QS : (r + 1) * QS]),
            "k": kpad[b],
            "v": vpad[b],
            "wq": wq,
            "wk": wk,
            "wv": wv,
            "wo": wo,
            "gw": gw,
            "gb": colmajor(gb),
            "kmb": np.ascontiguousarray(kmb[b].reshape(NJ, P).T),
            "qm": qmask[b, r * QS : (r + 1) * QS].astype(np.float32)[None, :],
        }
        if use_gamma:
            m["qg"], m["kg"], m["vg"] = (colmajor(g) for g in gams)
        if use_beta:
            m["qb"], m["kb"], m["vb"] = (colmajor(bt) for bt in bets)
        in_maps.append(m)
    return in_maps, (use_gamma, use_beta)


def kernel(_return_res=False, _run_kwargs=None, **inputs):
    run_kwargs = _run_kwargs or {}
    in_maps, key = make_in_maps(inputs)
    if key not in _CACHE:
        _CACHE[key] = _build(*key)
    nc = _CACHE[key]
    res = run_bass_kernel_spmd(nc, in_maps, list(range(NCORES)), **run_kwargs)
    out = np.empty((B, Q, D), np.float32)
    per_batch = NCORES // B
    for c in range(NCORES):
        b, r = c // per_batch, c % per_batch
        out[b, r * QS : (r + 1) * QS] = res.results[c]["out"]
    if _return_res:
        return out, res
    return out



# revision 5
# speedup vs baseline: 1.5082x; 1.5082x over previous
"""Trainium2 Bass kernel for BaseAttnPredictNet (pre-LN MHA with zero-attn
slot, gated output combination, residual).

Sharding: data-parallel over (batch, query-rows); 8 cores, 512 q rows each.

Host-side prep (layout only, no math): keys with mask==0 are dropped per
batch (attention is permutation-invariant over keys) and a zero-attn slot
appended; query rows are permuted active-first per core so attention runs
only on the first QA columns; weights are cast fp8 and pre-interleaved for
DoubleRow matmuls; the gate's query operand is pre-transposed.

On-device: LN in natural layout (DVE stats + Pool normalize), transposes
via the HWDGE DMA crossbar (no PE transposes), fp8 DoubleRow projections,
plain-fp8 64-contraction scores, softmax without max-subtraction
(exp(s*0.125 - 1), fp8 output), PV as per-head DoubleRow matmuls producing
transposed attention output plus a separate ones-matmul for denominators
(pad keys excluded via a 0/1 stationary), division folded with the query
mask, plain-fp8 output projection, DoubleRow gate, combine in natural
layout.
"""

import numpy as np
import ml_dtypes

import concourse.bass as bass
import concourse.bacc as bacc
import concourse.mybir as mybir
import concourse.tile as tile
from concourse.bass_utils import run_bass_kernel_spmd

P = 128
D = 512
H = 8
DH = 64
B, Q, KLEN = 2, 2048, 2048
QS = 512
NCORES = 8
PB = NCORES // B
SCALE = 0.125
LN_EPS = 1e-5

F32 = mybir.dt.float32
BF16 = mybir.dt.bfloat16
F8 = mybir.dt.float8e4
AF = mybir.ActivationFunctionType
OP = mybir.AluOpType
DRM = mybir.MatmulPerfMode.DoubleRow

NPF8 = ml_dtypes.float8_e4m3
NPBF = ml_dtypes.bfloat16


def _build(NJC: int, QA: int) -> bass.Bass:
    KPC = NJC * P
    NQA = QA // P
    NPR = NJC // 2
    TAIL = NJC - 2 * NPR

    nc = bacc.Bacc("TRN2", target_bir_lowering=False, debug=False)

    din = {}
    for name, shape, dt in (
        ("q", [QS, D], F32),
        ("qt", [2, P, 2, D], F8),
        ("k", [KPC, D], BF16),
        ("v", [KPC, D], BF16),
        ("wq", [2, P, 2, D], F8),
        ("wk", [2, P, 2, D], F8),
        ("wv", [2, P, 2, D], F8),
        ("wo", [DH, H, D], F8),
        ("gw", [4, P, 2, D], F8),
        ("gb", [P, 4], F32),
        ("kones", [P, NJC], F8),
        ("qm", [DH, QS], F32),
    ):
        din[name] = nc.dram_tensor(name, shape, dt, kind="ExternalInput")
    out_d = nc.dram_tensor("out", [QS, D], F32, kind="ExternalOutput")

    with tile.TileContext(nc) as tc:
        _body(nc, tc, din, out_d, NJC, QA, KPC, NQA, NPR, TAIL)
    nc.compile()
    return nc


def _body(nc, tc, din, out_d, NJC, QA, KPC, NQA, NPR, TAIL):
    from contextlib import ExitStack

    ctx = ExitStack()
    with ctx:
        persist = ctx.enter_context(tc.tile_pool(name="persist", bufs=1))
        stage = ctx.enter_context(tc.tile_pool(name="stage", bufs=6))
        stats = ctx.enter_context(tc.tile_pool(name="stats", bufs=8))
        nbuf = ctx.enter_context(tc.tile_pool(name="nbuf", bufs=4))
        pexp = ctx.enter_context(tc.tile_pool(name="pexp", bufs=2))
        prec = ctx.enter_context(tc.tile_pool(name="prec", bufs=4))
        cmb = ctx.enter_context(tc.tile_pool(name="cmb", bufs=6))
        # PSUM: 2 + 2*2 + 2 = 8 banks
        pacc = ctx.enter_context(tc.tile_pool(name="pacc", bufs=2, space="PSUM"))
        pS = ctx.enter_context(tc.tile_pool(name="pS", bufs=2, space="PSUM"))
        pnd = ctx.enter_context(tc.tile_pool(name="pnd", bufs=1, space="PSUM"))

        # ---- persistent inputs ----
        eps_t = persist.tile([P, 1], F32)
        nc.vector.memset(eps_t, LN_EPS)
        negone_t = persist.tile([P, 1], F32)
        nc.vector.memset(negone_t, -1.0)
        wq_t = persist.tile([P, 2, 2, D], F8, name="wq_t")
        wk_t = persist.tile([P, 2, 2, D], F8, name="wk_t")
        wv_t = persist.tile([P, 2, 2, D], F8, name="wv_t")
        for wt, wn in ((wq_t, "wq"), (wk_t, "wk"), (wv_t, "wv")):
            nc.sync.dma_start(out=wt, in_=din[wn][...].rearrange("j p i d -> p j i d"))
        wo_t = persist.tile([DH, H, D], F8, name="wo_t")
        nc.sync.dma_start(out=wo_t, in_=din["wo"][...])
        gw_t = persist.tile([P, 4, 2, D], F8, name="gw_t")
        nc.sync.dma_start(out=gw_t, in_=din["gw"][...].rearrange("j p i d -> p j i d"))
        gb_t = persist.tile([P, 4], F32, name="gb_t")
        nc.sync.dma_start(out=gb_t, in_=din["gb"][...])
        qt_t = persist.tile([P, 2, 2, D], F8, name="qt_t")
        nc.sync.dma_start(out=qt_t, in_=din["qt"][...].rearrange("j p i d -> p j i d"))
        kones_t = persist.tile([P, NJC], F8, name="kones_t")
        nc.sync.dma_start(out=kones_t, in_=din["kones"][...])
        qm_t = persist.tile([DH, QS], F32, name="qm_t")
        nc.sync.dma_start(out=qm_t, in_=din["qm"][...])
        q_nat = persist.tile([P, 4, D], F32, name="q_nat")
        nc.sync.dma_start(out=q_nat, in_=din["q"][...].rearrange("(a p) d -> p a d", p=P))

        kones_mat = persist.tile([P, NJC, DH], F8, name="kones_mat")
        nc.gpsimd.tensor_copy(
            kones_mat, kones_t[...].unsqueeze(2).broadcast_to((P, NJC, DH))
        )

        # ---- persistent activations ----
        qn_bf = persist.tile([P, 4, D], BF16, name="qn_bf")
        qnT_bf = persist.tile([P, 4, NQA * P], BF16, name="qnT_bf")
        qnT_f8 = persist.tile([P, 4, NQA * P], F8, name="qnT_f8")
        qhT = persist.tile([P, 4, QA], F8, name="qhT")
        knT_bf = persist.tile([P, 4, KPC], BF16, name="knT_bf")
        knT_f8 = persist.tile([P, 4, KPC], F8, name="knT_f8")
        khT = persist.tile([P, 4, KPC], F8, name="khT")
        vnT_bf = persist.tile([P, 4, KPC], BF16, name="vnT_bf")
        vnT_f8 = persist.tile([P, 4, KPC], F8, name="vnT_f8")
        vh_st = persist.tile([P, NJC, H, DH], F8, name="vh_st")
        av_t = persist.tile([DH, H, QS], F8, name="av_t")
        poT_f8 = persist.tile([P, 4, D], F8, name="poT_f8")
        poT_bf = persist.tile([P, 4, D], BF16, name="poT_bf")
        gT_bf = persist.tile([P, 4, D], BF16, name="gT_bf")
        po_nat = persist.tile([P, 4, D], BF16, name="po_nat")
        g_nat = persist.tile([P, 4, D], BF16, name="g_nat")
        out_nat = persist.tile([P, 4, D], F32, name="out_nat")

        nc.vector.memset(poT_f8, 0.0)
        nc.vector.memset(poT_bf, 0.0)

        def ln_scales(xf, cw):
            """(nm2, rstd) [P, cw] for chunk [P, cw, D]."""
            mv = stats.tile([P, 2, 2], F32, name="bnagg")
            for cc in range(cw):
                st = stats.tile([P, 6], F32, name="bnst")
                nc.vector.bn_stats(out=st, in_=xf[:, cc, :])
                nc.vector.bn_aggr(out=mv[:, cc, :], in_=st)
            std = stats.tile([P, 2], F32, name="std")
            nc.scalar.activation(
                out=std[:, :cw], in_=mv[:, 0:cw, 1], func=AF.Sqrt, bias=eps_t
            )
            rstd = stats.tile([P, 2], F32, name="rstd")
            nc.vector.reciprocal(rstd[:, :cw], std[:, :cw])
            nm2 = stats.tile([P, 2], F32, name="nm2")
            nc.vector.tensor_tensor(
                out=nm2[:, :cw], in0=mv[:, 0:cw, 0], in1=rstd[:, :cw], op=OP.mult
            )
            nc.vector.tensor_scalar_mul(nm2[:, :cw], nm2[:, :cw], -1.0)
            return nm2, rstd

        # ---- q: LN -> bf16 -> DMA-transpose (active blocks) -> fp8 ----
        for a0 in range(0, 4, 2):
            nm2, rstd = ln_scales(q_nat[:, a0 : a0 + 2, :], 2)
            for cc in range(2):
                a = a0 + cc
                eng = nc.gpsimd if a % 2 == 0 else nc.vector
                eng.tensor_scalar(
                    out=qn_bf[:, a, :],
                    in0=q_nat[:, a, :],
                    scalar1=nm2[:, cc : cc + 1],
                    scalar2=rstd[:, cc : cc + 1],
                    op0=OP.add,
                    op1=OP.mult,
                )
        for a in range(NQA):
            nc.sync.dma_start(
                out=qnT_bf[:, :, a * P : (a + 1) * P], in_=qn_bf[:, a, :],
                transpose=True,
            )
        nc.vector.tensor_copy(qnT_f8, qnT_bf)

        # ---- q proj (DoubleRow) ----
        for a in range(4):
            pp = pacc.tile([P, D], F32, name="pacc_t")
            for j in range(2):
                nc.tensor.matmul(
                    pp[:, 0:QA],
                    wq_t[:, j, :, a * P : (a + 1) * P],
                    qnT_f8[:, 2 * j : 2 * j + 2, 0:QA],
                    start=(j == 0),
                    stop=(j == 1),
                    perf_mode=DRM,
                )
            nc.vector.tensor_copy(qhT[:, a, :], pp[:, 0:QA])

        def ln_T_cast(src_dram, nT_bf, nT_f8, norm_eng):
            """k/v: DMA chunks -> LN -> bf16 -> DMA-transpose -> fp8 cast."""
            for c0 in range(0, NJC, 2):
                cw = min(2, NJC - c0)
                xst = stage.tile([P, 2, D], BF16, name="xst")
                nc.sync.dma_start(
                    out=xst[:, :cw, :],
                    in_=src_dram[c0 * P : (c0 + cw) * P, :].rearrange(
                        "(c p) d -> p c d", p=P
                    ),
                )
                nm2, rstd = ln_scales(xst, cw)
                xn = nbuf.tile([P, 2, D], BF16, name="xn")
                for cc in range(cw):
                    norm_eng.tensor_scalar(
                        out=xn[:, cc, :],
                        in0=xst[:, cc, :],
                        scalar1=nm2[:, cc : cc + 1],
                        scalar2=rstd[:, cc : cc + 1],
                        op0=OP.add,
                        op1=OP.mult,
                    )
                for cc in range(cw):
                    c = c0 + cc
                    nc.sync.dma_start(
                        out=nT_bf[:, :, c * P : (c + 1) * P], in_=xn[:, cc, :],
                        transpose=True,
                    )
                nc.vector.tensor_copy(
                    nT_f8[:, :, c0 * P : (c0 + cw) * P],
                    nT_bf[:, :, c0 * P : (c0 + cw) * P],
                )

        # ---- k path + k proj ----
        ln_T_cast(din["k"], knT_bf, knT_f8, nc.gpsimd)
        for a in range(4):
            for n0 in range(0, KPC, 512):
                nw = min(512, KPC - n0)
                pp = pacc.tile([P, D], F32, name="pacc_t")
                for j in range(2):
                    nc.tensor.matmul(
                        pp[:, :nw],
                        wk_t[:, j, :, a * P : (a + 1) * P],
                        knT_f8[:, 2 * j : 2 * j + 2, n0 : n0 + nw],
                        start=(j == 0),
                        stop=(j == 1),
                        perf_mode=DRM,
                    )
                nc.vector.tensor_copy(khT[:, a, n0 : n0 + nw], pp[:, :nw])

        # ---- v path + v proj into vh_st ----
        ln_T_cast(din["v"], vnT_bf, vnT_f8, nc.gpsimd)
        for c in range(NJC):
            pp = pacc.tile([P, D], F32, name="pacc_t")
            for j in range(2):
                nc.tensor.matmul(
                    pp,
                    vnT_f8[:, 2 * j : 2 * j + 2, c * P : (c + 1) * P],
                    wv_t[:, j, :, :],
                    start=(j == 0),
                    stop=(j == 1),
                    perf_mode=DRM,
                )
            nc.scalar.copy(
                vh_st[:, c, :, :], pp[...].rearrange("p (h e) -> p h e", h=H)
            )

        # ---- attention heads ----
        for h in range(H):
            nb = h // 2
            r0 = (h % 2) * DH
            expS = pexp.tile([P, NJC, QA], F8, name="expS")
            for c0 in range(0, NJC, 2):
                cw = min(2, NJC - c0)
                ps = pS.tile([P, 2, 512], F32, name="pS_t")
                for i in range(cw):
                    c = c0 + i
                    nc.tensor.matmul(
                        ps[:, i, 0:QA],
                        khT[r0 : r0 + DH, nb, c * P : (c + 1) * P],
                        qhT[r0 : r0 + DH, nb, :],
                        start=True,
                        stop=True,
                    )
                nc.scalar.activation(
                    out=expS[:, c0 : c0 + cw, :],
                    in_=ps[:, 0:cw, 0:QA],
                    func=AF.Exp,
                    scale=SCALE,
                    bias=negone_t,
                )
            pnum = pnd.tile([DH, 512], F32, name="pnum")
            pden = pnd.tile([DH, 512], F32, name="pden")
            for pr in range(NPR):
                fl = dict(start=(pr == 0), stop=(TAIL == 0 and pr == NPR - 1))
                nc.tensor.matmul(
                    pnum[:, 0:QA],
                    vh_st[:, 2 * pr : 2 * pr + 2, h, :],
                    expS[:, 2 * pr : 2 * pr + 2, :],
                    perf_mode=DRM,
                    **fl,
                )
                nc.tensor.matmul(
                    pden[:, 0:QA],
                    kones_mat[:, 2 * pr : 2 * pr + 2, :],
                    expS[:, 2 * pr : 2 * pr + 2, :],
                    perf_mode=DRM,
                    **fl,
                )
            if TAIL:
                nc.tensor.matmul(
                    pnum[:, 0:QA], vh_st[:, NJC - 1, h, :], expS[:, NJC - 1, :],
                    start=(NPR == 0), stop=True,
                )
                nc.tensor.matmul(
                    pden[:, 0:QA], kones_mat[:, NJC - 1, :], expS[:, NJC - 1, :],
                    start=(NPR == 0), stop=True,
                )
            rec = prec.tile([DH, QA], F32, name="rec")
            nc.vector.reciprocal(rec, pden[:, 0:QA])
            rec2 = prec.tile([DH, QA], F32, name="rec2")
            nc.vector.tensor_tensor(
                out=rec2, in0=rec, in1=qm_t[:, 0:QA], op=OP.mult
            )
            nc.vector.tensor_tensor(
                out=av_t[:, h, 0:QA], in0=pnum[:, 0:QA], in1=rec2, op=OP.mult
            )

        # ---- output projection (plain fp8, contraction 64 per head) ----
        for a in range(4):
            pp = pacc.tile([P, D], F32, name="pacc_t")
            for h in range(H):
                nc.tensor.matmul(
                    pp[:, 0:QA],
                    wo_t[:, h, a * P : (a + 1) * P],
                    av_t[:, h, 0:QA],
                    start=(h == 0),
                    stop=(h == H - 1),
                )
            nc.scalar.copy(poT_f8[:, a, 0:QA], pp[:, 0:QA])
            nc.vector.tensor_copy(poT_bf[:, a, 0:QA], pp[:, 0:QA])

        # ---- gate (DoubleRow over [q; po], K=1024) ----
        for a in range(4):
            pp = pacc.tile([P, D], F32, name="pacc_t")
            for j in range(4):
                rhs = (
                    qt_t[:, j, :, :]
                    if j < 2
                    else poT_f8[:, 2 * (j - 2) : 2 * (j - 2) + 2, :]
                )
                nc.tensor.matmul(
                    pp,
                    gw_t[:, j, :, a * P : (a + 1) * P],
                    rhs,
                    start=(j == 0),
                    stop=(j == 3),
                    perf_mode=DRM,
                )
            nc.scalar.activation(
                out=gT_bf[:, a, :], in_=pp, func=AF.Sigmoid, bias=gb_t[:, a : a + 1]
            )

        # ---- back to natural layout + combine ----
        for a in range(4):
            nc.sync.dma_start(
                out=po_nat[:, :, a * P : (a + 1) * P], in_=poT_bf[:, a, :],
                transpose=True,
            )
            nc.sync.dma_start(
                out=g_nat[:, :, a * P : (a + 1) * P], in_=gT_bf[:, a, :],
                transpose=True,
            )
        for a in range(4):
            s = cmb.tile([P, D], F32, name="cmb_t")
            nc.vector.tensor_tensor(
                out=s, in0=q_nat[:, a, :], in1=po_nat[:, a, :], op=OP.subtract
            )
            r = cmb.tile([P, D], F32, name="cmb_t")
            nc.gpsimd.tensor_tensor(
                out=r, in0=q_nat[:, a, :], in1=po_nat[:, a, :], op=OP.add
            )
            m = cmb.tile([P, D], F32, name="cmb_t")
            nc.vector.tensor_tensor(out=m, in0=g_nat[:, a, :], in1=s, op=OP.mult)
            nc.gpsimd.tensor_tensor(out=out_nat[:, a, :], in0=m, in1=r, op=OP.add)

        nc.sync.dma_start(
            out=out_d[:, :].rearrange("(a p) d -> p a d", p=P), in_=out_nat
        )


_CACHE: dict = {}


def make_in_maps(inputs):
    q = np.asarray(inputs["query"], np.float32)
    k = np.asarray(inputs["key"], np.float32)
    v = np.asarray(inputs["value"], np.float32)
    wq = np.asarray(inputs["weight_q"], np.float32)
    wk = np.asarray(inputs["weight_k"], np.float32)
    wv = np.asarray(inputs["weight_v"], np.float32)
    wo = np.asarray(inputs["weight_o"], np.float32)
    gw = np.asarray(inputs["g_w"], np.float32)
    gb = np.asarray(inputs["g_b"], np.float32)
    qmask = np.asarray(inputs["query_mask"])
    kmask = np.asarray(inputs["key_mask"])
    gams = {n: np.asarray(inputs[n], np.float32) for n in ("q_gamma", "k_gamma", "v_gamma")}
    bets = [np.asarray(inputs[n], np.float32) for n in ("q_beta", "k_beta", "v_beta")]
    if any(np.any(bt != 0.0) for bt in bets):
        raise NotImplementedError("nonzero LN beta not supported")

    # gamma folds into the projection weights: (z*g) @ W == z @ (diag(g) W)
    wq = gams["q_gamma"][:, None] * wq
    wk = gams["k_gamma"][:, None] * wk
    wv = gams["v_gamma"][:, None] * wv

    def dr4(w):  # [D, D] -> [2, 128, 2, D] DoubleRow-interleaved, fp8
        return np.ascontiguousarray(
            w.reshape(2, 2, P, D).transpose(0, 2, 1, 3)
        ).astype(NPF8)

    wq8, wk8, wv8 = dr4(wq), dr4(wk), dr4(wv)
    wo8 = np.ascontiguousarray(wo.reshape(H, DH, D).transpose(1, 0, 2)).astype(NPF8)
    gw8 = np.ascontiguousarray(
        gw.reshape(4, 2, P, D).transpose(0, 2, 1, 3)
    ).astype(NPF8)
    gb_cm = np.ascontiguousarray(gb.reshape(4, P).T)

    # key compaction: keep mask!=0, append zero-attn slot, pad to NJC*128
    kept = [np.nonzero(kmask[b])[0] for b in range(B)]
    nkp = [len(ix) + 1 for ix in kept]
    NJC = max(1, (max(nkp) + P - 1) // P)
    KPC = NJC * P
    k_in = np.zeros((B, KPC, D), NPBF)
    v_in = np.zeros((B, KPC, D), NPBF)
    kones = np.zeros((B, P, NJC), NPF8)
    for b in range(B):
        k_in[b, : nkp[b] - 1] = k[b, kept[b]].astype(NPBF)
        v_in[b, : nkp[b] - 1] = v[b, kept[b]].astype(NPBF)
        ar = np.zeros(KPC, np.float32)
        ar[: nkp[b]] = 1.0
        kones[b] = ar.reshape(NJC, P).T.astype(NPF8)

    # query rows: active-first permutation per core
    rows = []
    for b in range(B):
        act = np.nonzero(qmask[b])[0]
        inact = np.nonzero(qmask[b] == 0)[0]
        acts = [act[r::PB] for r in range(PB)]
        pos = 0
        for r in range(PB):
            need = QS - len(acts[r])
            rows.append((b, np.concatenate([acts[r], inact[pos : pos + need]])))
            pos += need
        assert pos == len(inact)
    max_act = max(int(np.sum(qmask[b][r] != 0)) for b, r in rows)
    QA = min(QS, max(P, ((max_act + P - 1) // P) * P))

    in_maps = []
    for c in range(NCORES):
        b, rw = rows[c]
        qc = np.ascontiguousarray(q[b, rw])
        qt8 = np.ascontiguousarray(
            qc.T.reshape(2, 2, P, QS).transpose(0, 2, 1, 3)
        ).astype(NPF8)
        qm_bc = np.broadcast_to(
            (qmask[b, rw] != 0).astype(np.float32)[None, :], (DH, QS)
        )
        in_maps.append(
            {
                "q": qc,
                "qt": qt8,
                "k": k_in[b],
                "v": v_in[b],
                "wq": wq8,
                "wk": wk8,
                "wv": wv8,
                "wo": wo8,
                "gw": gw8,
                "gb": gb_cm,
                "kones": kones[b],
                "qm": np.ascontiguousarray(qm_bc),
            }
        )
    return in_maps, rows, (NJC, QA)


def kernel(_return_res=False, _run_kwargs=None, **inputs):
    run_kwargs = _run_kwargs or {}
    in_maps, rows, key = make_in_maps(inputs)
    if key not in _CACHE:
        _CACHE[key] = _build(*key)
    nc = _CACHE[key]
    res = run_bass_kernel_spmd(nc, in_maps, list(range(NCORES)), **run_kwargs)
    out = np.empty((B, Q, D), np.float32)
    for c in range(NCORES):
        b, rw = rows[c]
        out[b, rw] = res.results[c]["out"]
    if _return_res:
        return out, res
    return out


# revision 8
# speedup vs baseline: 1.6013x; 1.0617x over previous
"""Trainium2 Bass kernel for BaseAttnPredictNet (pre-LN MHA with zero-attn
slot, gated output combination, residual).

Sharding: data-parallel over (batch, query-rows); 8 cores, 512 q rows each.

Host-side prep (layout only, no math): keys with mask==0 are dropped per
batch (attention is permutation-invariant over keys) and a zero-attn slot
appended; query rows are permuted active-first per core so attention runs
only on the first QA columns; weights are cast fp8 and pre-interleaved for
DoubleRow matmuls; the gate's query operand is pre-transposed.

On-device: LN in natural layout (DVE stats + Pool normalize), transposes
via the HWDGE DMA crossbar (no PE transposes), fp8 DoubleRow projections,
plain-fp8 64-contraction scores, softmax without max-subtraction
(exp(s*0.125 - 1), fp8 output), PV as per-head DoubleRow matmuls producing
transposed attention output plus a separate ones-matmul for denominators
(pad keys excluded via a 0/1 stationary), division folded with the query
mask, plain-fp8 output projection, DoubleRow gate, combine in natural
layout.
"""

import numpy as np
import ml_dtypes

import concourse.bass as bass
import concourse.bacc as bacc
import concourse.mybir as mybir
import concourse.tile as tile
from concourse.bass_utils import run_bass_kernel_spmd

P = 128
D = 512
H = 8
DH = 64
B, Q, KLEN = 2, 2048, 2048
QS = 512
NCORES = 8
PB = NCORES // B
SCALE = 0.125
LN_EPS = 1e-5

F32 = mybir.dt.float32
BF16 = mybir.dt.bfloat16
F8 = mybir.dt.float8e4
AF = mybir.ActivationFunctionType
OP = mybir.AluOpType
DRM = mybir.MatmulPerfMode.DoubleRow

NPF8 = ml_dtypes.float8_e4m3
NPBF = ml_dtypes.bfloat16


def _build(NJC: int, QA: int) -> bass.Bass:
    KPC = NJC * P
    NQA = QA // P
    NPR = NJC // 2
    TAIL = NJC - 2 * NPR

    nc = bacc.Bacc("TRN2", target_bir_lowering=False, debug=False)

    din = {}
    for name, shape, dt in (
        ("q", [QS, D], F32),
        ("qt", [2, P, 2, D], F8),
        ("k", [KPC, D], BF16),
        ("v", [KPC, D], BF16),
        ("wq", [2, P, 2, D], F8),
        ("wk", [2, P, 2, D], F8),
        ("wv", [2, P, 2, D], F8),
        ("wo", [DH, H, D], F8),
        ("gw", [4, P, 2, D], F8),
        ("gb", [P, 4], F32),
        ("kones", [P, NJC], F8),
        ("qm", [DH, QS], F32),
    ):
        din[name] = nc.dram_tensor(name, shape, dt, kind="ExternalInput")
    out_d = nc.dram_tensor("out", [QS, D], F32, kind="ExternalOutput")

    with tile.TileContext(nc) as tc:
        _body(nc, tc, din, out_d, NJC, QA, KPC, NQA, NPR, TAIL)
    nc.compile()
    return nc


def _body(nc, tc, din, out_d, NJC, QA, KPC, NQA, NPR, TAIL):
    from contextlib import ExitStack

    ctx = ExitStack()
    with ctx:
        persist = ctx.enter_context(tc.tile_pool(name="persist", bufs=1))
        stage = ctx.enter_context(tc.tile_pool(name="stage", bufs=6))
        stats = ctx.enter_context(tc.tile_pool(name="stats", bufs=8))
        nbuf = ctx.enter_context(tc.tile_pool(name="nbuf", bufs=4))
        pexp = ctx.enter_context(tc.tile_pool(name="pexp", bufs=2))
        prec = ctx.enter_context(tc.tile_pool(name="prec", bufs=4))
        cmb = ctx.enter_context(tc.tile_pool(name="cmb", bufs=6))
        # PSUM: 2 + 2*2 + 2 = 8 banks
        pacc = ctx.enter_context(tc.tile_pool(name="pacc", bufs=2, space="PSUM"))
        pS = ctx.enter_context(tc.tile_pool(name="pS", bufs=2, space="PSUM"))
        pnd = ctx.enter_context(tc.tile_pool(name="pnd", bufs=1, space="PSUM"))

        # ---- persistent inputs ----
        eps_t = persist.tile([P, 1], F32)
        nc.vector.memset(eps_t, LN_EPS)
        negone_t = persist.tile([P, 1], F32)
        nc.vector.memset(negone_t, -1.0)
        wq_t = persist.tile([P, 2, 2, D], F8, name="wq_t")
        wk_t = persist.tile([P, 2, 2, D], F8, name="wk_t")
        wv_t = persist.tile([P, 2, 2, D], F8, name="wv_t")
        for wt, wn in ((wq_t, "wq"), (wk_t, "wk"), (wv_t, "wv")):
            nc.sync.dma_start(out=wt, in_=din[wn][...].rearrange("j p i d -> p j i d"))
        wo_t = persist.tile([DH, H, D], F8, name="wo_t")
        nc.scalar.dma_start(out=wo_t, in_=din["wo"][...])
        gw_t = persist.tile([P, 4, 2, D], F8, name="gw_t")
        nc.sync.dma_start(out=gw_t, in_=din["gw"][...].rearrange("j p i d -> p j i d"))
        gb_t = persist.tile([P, 4], F32, name="gb_t")
        nc.sync.dma_start(out=gb_t, in_=din["gb"][...])
        qt_t = persist.tile([P, 2, 2, D], F8, name="qt_t")
        nc.scalar.dma_start(out=qt_t, in_=din["qt"][...].rearrange("j p i d -> p j i d"))
        kones_t = persist.tile([P, NJC], F8, name="kones_t")
        nc.sync.dma_start(out=kones_t, in_=din["kones"][...])
        qm_t = persist.tile([DH, QS], F32, name="qm_t")
        nc.sync.dma_start(out=qm_t, in_=din["qm"][...])
        q_nat = persist.tile([P, 4, D], F32, name="q_nat")
        nc.sync.dma_start(out=q_nat, in_=din["q"][...].rearrange("(a p) d -> p a d", p=P))

        kones_mat = persist.tile([P, NJC, DH], F8, name="kones_mat")
        nc.gpsimd.tensor_copy(
            kones_mat, kones_t[...].unsqueeze(2).broadcast_to((P, NJC, DH))
        )

        # ---- persistent activations ----
        qn_bf = persist.tile([P, 4, D], BF16, name="qn_bf")
        qnT_bf = persist.tile([P, 4, NQA * P], BF16, name="qnT_bf")
        qnT_f8 = persist.tile([P, 4, NQA * P], F8, name="qnT_f8")
        qhT = persist.tile([P, 4, QA], F8, name="qhT")
        knT_bf = persist.tile([P, 4, KPC], BF16, name="knT_bf")
        knT_f8 = persist.tile([P, 4, KPC], F8, name="knT_f8")
        khT = persist.tile([P, 4, KPC], F8, name="khT")
        vnT_bf = persist.tile([P, 4, KPC], BF16, name="vnT_bf")
        vnT_f8 = persist.tile([P, 4, KPC], F8, name="vnT_f8")
        vh_st = persist.tile([P, NJC, H, DH], F8, name="vh_st")
        av_t = persist.tile([DH, H, QS], F8, name="av_t")
        poT_f8 = persist.tile([P, 4, D], F8, name="poT_f8")
        poT_bf = persist.tile([P, 4, D], BF16, name="poT_bf")
        gT_bf = persist.tile([P, 4, D], BF16, name="gT_bf")
        po_nat = persist.tile([P, 4, D], BF16, name="po_nat")
        g_nat = persist.tile([P, 4, D], BF16, name="g_nat")
        out_nat = persist.tile([P, 4, D], F32, name="out_nat")

        nc.vector.memset(poT_f8, 0.0)
        nc.vector.memset(poT_bf, 0.0)

        def ln_scales(xf, cw):
            """(nm2, rstd) [P, cw] for chunk [P, cw, D]."""
            mv = stats.tile([P, 2, 2], F32, name="bnagg")
            for cc in range(cw):
                st = stats.tile([P, 6], F32, name="bnst")
                nc.vector.bn_stats(out=st, in_=xf[:, cc, :])
                nc.vector.bn_aggr(out=mv[:, cc, :], in_=st)
            std = stats.tile([P, 2], F32, name="std")
            nc.scalar.activation(
                out=std[:, :cw], in_=mv[:, 0:cw, 1], func=AF.Sqrt, bias=eps_t
            )
            rstd = stats.tile([P, 2], F32, name="rstd")
            nc.vector.reciprocal(rstd[:, :cw], std[:, :cw])
            nm2 = stats.tile([P, 2], F32, name="nm2")
            nc.vector.tensor_tensor(
                out=nm2[:, :cw], in0=mv[:, 0:cw, 0], in1=rstd[:, :cw], op=OP.mult
            )
            nc.vector.tensor_scalar_mul(nm2[:, :cw], nm2[:, :cw], -1.0)
            return nm2, rstd

        # ---- q: LN -> bf16 -> DMA-transpose (active blocks) -> fp8 ----
        for a0 in range(0, 4, 2):
            nm2, rstd = ln_scales(q_nat[:, a0 : a0 + 2, :], 2)
            for cc in range(2):
                a = a0 + cc
                eng = nc.gpsimd if a % 2 == 0 else nc.vector
                eng.tensor_scalar(
                    out=qn_bf[:, a, :],
                    in0=q_nat[:, a, :],
                    scalar1=nm2[:, cc : cc + 1],
                    scalar2=rstd[:, cc : cc + 1],
                    op0=OP.add,
                    op1=OP.mult,
                )
        for a in range(NQA):
            nc.scalar.dma_start(
                out=qnT_bf[:, :, a * P : (a + 1) * P], in_=qn_bf[:, a, :],
                transpose=True,
            )
        nc.vector.tensor_copy(qnT_f8, qnT_bf)

        # ---- q proj (DoubleRow) ----
        for a in range(4):
            pp = pacc.tile([P, D], F32, name="pacc_t")
            for j in range(2):
                nc.tensor.matmul(
                    pp[:, 0:QA],
                    wq_t[:, j, :, a * P : (a + 1) * P],
                    qnT_f8[:, 2 * j : 2 * j + 2, 0:QA],
                    start=(j == 0),
                    stop=(j == 1),
                    perf_mode=DRM,
                )
            nc.vector.tensor_copy(qhT[:, a, :], pp[:, 0:QA])

        def ln_T_cast(src_dram, nT_bf, nT_f8, norm_eng, dma_eng):
            """k/v: DMA chunks -> LN -> bf16 -> DMA-transpose -> fp8 cast."""
            for c0 in range(0, NJC, 2):
                cw = min(2, NJC - c0)
                xst = stage.tile([P, 2, D], BF16, name="xst")
                dma_eng.dma_start(
                    out=xst[:, :cw, :],
                    in_=src_dram[c0 * P : (c0 + cw) * P, :].rearrange(
                        "(c p) d -> p c d", p=P
                    ),
                )
                nm2, rstd = ln_scales(xst, cw)
                xn = nbuf.tile([P, 2, D], BF16, name="xn")
                for cc in range(cw):
                    norm_eng.tensor_scalar(
                        out=xn[:, cc, :],
                        in0=xst[:, cc, :],
                        scalar1=nm2[:, cc : cc + 1],
                        scalar2=rstd[:, cc : cc + 1],
                        op0=OP.add,
                        op1=OP.mult,
                    )
                for cc in range(cw):
                    c = c0 + cc
                    dma_eng.dma_start(
                        out=nT_bf[:, :, c * P : (c + 1) * P], in_=xn[:, cc, :],
                        transpose=True,
                    )
                nc.vector.tensor_copy(
                    nT_f8[:, :, c0 * P : (c0 + cw) * P],
                    nT_bf[:, :, c0 * P : (c0 + cw) * P],
                )

        # ---- k path + k proj ----
        ln_T_cast(din["k"], knT_bf, knT_f8, nc.gpsimd, nc.sync)
        for a in range(4):
            for n0 in range(0, KPC, 512):
                nw = min(512, KPC - n0)
                pp = pacc.tile([P, D], F32, name="pacc_t")
                for j in range(2):
                    nc.tensor.matmul(
                        pp[:, :nw],
                        wk_t[:, j, :, a * P : (a + 1) * P],
                        knT_f8[:, 2 * j : 2 * j + 2, n0 : n0 + nw],
                        start=(j == 0),
                        stop=(j == 1),
                        perf_mode=DRM,
                    )
                nc.vector.tensor_copy(khT[:, a, n0 : n0 + nw], pp[:, :nw])

        # ---- v path + v proj into vh_st ----
        ln_T_cast(din["v"], vnT_bf, vnT_f8, nc.gpsimd, nc.scalar)
        for c in range(NJC):
            pp = pacc.tile([P, D], F32, name="pacc_t")
            for j in range(2):
                nc.tensor.matmul(
                    pp,
                    vnT_f8[:, 2 * j : 2 * j + 2, c * P : (c + 1) * P],
                    wv_t[:, j, :, :],
                    start=(j == 0),
                    stop=(j == 1),
                    perf_mode=DRM,
                )
            nc.scalar.copy(
                vh_st[:, c, :, :], pp[...].rearrange("p (h e) -> p h e", h=H)
            )

        # ---- attention heads ----
        for h in range(H):
            nb = h // 2
            r0 = (h % 2) * DH
            expS = pexp.tile([P, NJC, QA], F8, name="expS")
            for c0 in range(0, NJC, 2):
                cw = min(2, NJC - c0)
                ps = pS.tile([P, 2, 512], F32, name="pS_t")
                for i in range(cw):
                    c = c0 + i
                    nc.tensor.matmul(
                        ps[:, i, 0:QA],
                        khT[r0 : r0 + DH, nb, c * P : (c + 1) * P],
                        qhT[r0 : r0 + DH, nb, :],
                        start=True,
                        stop=True,
                    )
                nc.scalar.activation(
                    out=expS[:, c0 : c0 + cw, :],
                    in_=ps[:, 0:cw, 0:QA],
                    func=AF.Exp,
                    scale=SCALE,
                    bias=negone_t,
                )
            pnum = pnd.tile([DH, 512], F32, name="pnum")
            pden = pnd.tile([DH, 512], F32, name="pden")
            for pr in range(NPR):
                fl = dict(start=(pr == 0), stop=(TAIL == 0 and pr == NPR - 1))
                nc.tensor.matmul(
                    pnum[:, 0:QA],
                    vh_st[:, 2 * pr : 2 * pr + 2, h, :],
                    expS[:, 2 * pr : 2 * pr + 2, :],
                    perf_mode=DRM,
                    **fl,
                )
                nc.tensor.matmul(
                    pden[:, 0:QA],
                    kones_mat[:, 2 * pr : 2 * pr + 2, :],
                    expS[:, 2 * pr : 2 * pr + 2, :],
                    perf_mode=DRM,
                    **fl,
                )
            if TAIL:
                nc.tensor.matmul(
                    pnum[:, 0:QA], vh_st[:, NJC - 1, h, :], expS[:, NJC - 1, :],
                    start=(NPR == 0), stop=True,
                )
                nc.tensor.matmul(
                    pden[:, 0:QA], kones_mat[:, NJC - 1, :], expS[:, NJC - 1, :],
                    start=(NPR == 0), stop=True,
                )
            rec = prec.tile([DH, QA], F32, name="rec")
            nc.vector.reciprocal_approx_fast(out=rec, in_=pden[:, 0:QA])
            rec2 = prec.tile([DH, QA], F32, name="rec2")
            nc.vector.tensor_tensor(
                out=rec2, in0=rec, in1=qm_t[:, 0:QA], op=OP.mult
            )
            nc.vector.tensor_tensor(
                out=av_t[:, h, 0:QA], in0=pnum[:, 0:QA], in1=rec2, op=OP.mult
            )

        # ---- output projection (plain fp8, contraction 64 per head) ----
        for a in range(4):
            pp = pacc.tile([P, D], F32, name="pacc_t")
            for h in range(H):
                nc.tensor.matmul(
                    pp[:, 0:QA],
                    wo_t[:, h, a * P : (a + 1) * P],
                    av_t[:, h, 0:QA],
                    start=(h == 0),
                    stop=(h == H - 1),
                )
            nc.scalar.copy(poT_f8[:, a, 0:QA], pp[:, 0:QA])
            nc.vector.tensor_copy(poT_bf[:, a, 0:QA], pp[:, 0:QA])

        # ---- gate (DoubleRow over [q; po], K=1024) ----
        for a in range(4):
            pp = pacc.tile([P, D], F32, name="pacc_t")
            for j in range(4):
                rhs = (
                    qt_t[:, j, :, :]
                    if j < 2
                    else poT_f8[:, 2 * (j - 2) : 2 * (j - 2) + 2, :]
                )
                nc.tensor.matmul(
                    pp,
                    gw_t[:, j, :, a * P : (a + 1) * P],
                    rhs,
                    start=(j == 0),
                    stop=(j == 3),
                    perf_mode=DRM,
                )
            nc.scalar.activation(
                out=gT_bf[:, a, :], in_=pp, func=AF.Sigmoid, bias=gb_t[:, a : a + 1]
            )

        # ---- back to natural layout + combine ----
        for a in range(4):
            nc.scalar.dma_start(
                out=po_nat[:, :, a * P : (a + 1) * P], in_=poT_bf[:, a, :],
                transpose=True,
            )
            nc.sync.dma_start(
                out=g_nat[:, :, a * P : (a + 1) * P], in_=gT_bf[:, a, :],
                transpose=True,
            )
        for a in range(4):
            s = cmb.tile([P, D], F32, name="cmb_t")
            nc.vector.tensor_tensor(
                out=s, in0=q_nat[:, a, :], in1=po_nat[:, a, :], op=OP.subtract
            )
            r = cmb.tile([P, D], F32, name="cmb_t")
            nc.gpsimd.tensor_tensor(
                out=r, in0=q_nat[:, a, :], in1=po_nat[:, a, :], op=OP.add
            )
            m = cmb.tile([P, D], F32, name="cmb_t")
            nc.vector.tensor_tensor(out=m, in0=g_nat[:, a, :], in1=s, op=OP.mult)
            nc.gpsimd.tensor_tensor(out=out_nat[:, a, :], in0=m, in1=r, op=OP.add)

        nc.sync.dma_start(
            out=out_d[:, :].rearrange("(a p) d -> p a d", p=P), in_=out_nat
        )


_CACHE: dict = {}


def make_in_maps(inputs):
    q = np.asarray(inputs["query"], np.float32)
    k = np.asarray(inputs["key"], np.float32)
    v = np.asarray(inputs["value"], np.float32)
    wq = np.asarray(inputs["weight_q"], np.float32)
    wk = np.asarray(inputs["weight_k"], np.float32)
    wv = np.asarray(inputs["weight_v"], np.float32)
    wo = np.asarray(inputs["weight_o"], np.float32)
    gw = np.asarray(inputs["g_w"], np.float32)
    gb = np.asarray(inputs["g_b"], np.float32)
    qmask = np.asarray(inputs["query_mask"])
    kmask = np.asarray(inputs["key_mask"])
    gams = {n: np.asarray(inputs[n], np.float32) for n in ("q_gamma", "k_gamma", "v_gamma")}
    bets = [np.asarray(inputs[n], np.float32) for n in ("q_beta", "k_beta", "v_beta")]
    if any(np.any(bt != 0.0) for bt in bets):
        raise NotImplementedError("nonzero LN beta not supported")

    # gamma folds into the projection weights: (z*g) @ W == z @ (diag(g) W)
    wq = gams["q_gamma"][:, None] * wq
    wk = gams["k_gamma"][:, None] * wk
    wv = gams["v_gamma"][:, None] * wv

    def dr4(w):  # [D, D] -> [2, 128, 2, D] DoubleRow-interleaved, fp8
        return np.ascontiguousarray(
            w.reshape(2, 2, P, D).transpose(0, 2, 1, 3)
        ).astype(NPF8)

    wq8, wk8, wv8 = dr4(wq), dr4(wk), dr4(wv)
    wo8 = np.ascontiguousarray(wo.reshape(H, DH, D).transpose(1, 0, 2)).astype(NPF8)
    gw8 = np.ascontiguousarray(
        gw.reshape(4, 2, P, D).transpose(0, 2, 1, 3)
    ).astype(NPF8)
    gb_cm = np.ascontiguousarray(gb.reshape(4, P).T)

    # key compaction: keep mask!=0, append zero-attn slot, pad to NJC*128
    kept = [np.nonzero(kmask[b])[0] for b in range(B)]
    nkp = [len(ix) + 1 for ix in kept]
    NJC = max(1, (max(nkp) + P - 1) // P)
    KPC = NJC * P
    k_in = np.zeros((B, KPC, D), NPBF)
    v_in = np.zeros((B, KPC, D), NPBF)
    kones = np.zeros((B, P, NJC), NPF8)
    for b in range(B):
        k_in[b, : nkp[b] - 1] = k[b, kept[b]].astype(NPBF)
        v_in[b, : nkp[b] - 1] = v[b, kept[b]].astype(NPBF)
        ar = np.zeros(KPC, np.float32)
        ar[: nkp[b]] = 1.0
        kones[b] = ar.reshape(NJC, P).T.astype(NPF8)

    # query rows: active-first permutation per core
    rows = []
    for b in range(B):
        act = np.nonzero(qmask[b])[0]
        inact = np.nonzero(qmask[b] == 0)[0]
        acts = [act[r::PB] for r in range(PB)]
        pos = 0
        for r in range(PB):
            need = QS - len(acts[r])
            rows.append((b, np.concatenate([acts[r], inact[pos : pos + need]])))
            pos += need
        assert pos == len(inact)
    max_act = max(int(np.sum(qmask[b][r] != 0)) for b, r in rows)
    QA = min(QS, max(P, ((max_act + P - 1) // P) * P))

    in_maps = []
    for c in range(NCORES):
        b, rw = rows[c]
        qc = np.ascontiguousarray(q[b, rw])
        qt8 = np.ascontiguousarray(
            qc.T.reshape(2, 2, P, QS).transpose(0, 2, 1, 3)
        ).astype(NPF8)
        qm_bc = np.broadcast_to(
            (qmask[b, rw] != 0).astype(np.float32)[None, :], (DH, QS)
        )
        in_maps.append(
            {
                "q": qc,
                "qt": qt8,
                "k": k_in[b],
                "v": v_in[b],
                "wq": wq8,
                "wk": wk8,
                "wv": wv8,
                "wo": wo8,
                "gw": gw8,
                "gb": gb_cm,
                "kones": kones[b],
                "qm": np.ascontiguousarray(qm_bc),
            }
        )
    return in_maps, rows, (NJC, QA)


def kernel(_return_res=False, _run_kwargs=None, **inputs):
    run_kwargs = _run_kwargs or {}
    in_maps, rows, key = make_in_maps(inputs)
    if key not in _CACHE:
        _CACHE[key] = _build(*key)
    nc = _CACHE[key]
    res = run_bass_kernel_spmd(nc, in_maps, list(range(NCORES)), **run_kwargs)
    out = np.empty((B, Q, D), np.float32)
    for c in range(NCORES):
        b, rw = rows[c]
        out[b, rw] = res.results[c]["out"]
    if _return_res:
        return out, res
    return out


# revision 9
# speedup vs baseline: 1.9007x; 1.1870x over previous
"""Trainium2 Bass kernel for BaseAttnPredictNet (pre-LN MHA with zero-attn
slot, gated output combination, residual).

Sharding: data-parallel over (batch, query-rows); 8 cores, 512 q rows each.

Host-side prep (layout only, no math): keys with mask==0 are dropped per
batch (attention is permutation-invariant over keys) and a zero-attn slot
appended; query rows are permuted active-first per core so attention runs
only on the first QA columns; weights are cast fp8 and pre-interleaved for
DoubleRow matmuls; the gate's query operand is pre-transposed.

On-device: LN in natural layout (batched DVE stats, Pool normalize),
transposes via the HWDGE DMA crossbar (no PE transposes), fp8 DoubleRow
projections, plain-fp8 64-contraction scores interleaved across head pairs
on opposite PE row-tiles, softmax without max-subtraction (exp(s/8 - 1),
fp8 out), PV as per-head DoubleRow matmuls producing transposed attention
output plus a ones-matmul for denominators (pad keys excluded via a 0/1
stationary), division folded with the query mask, plain-fp8 output
projection, DoubleRow gate, bf16 combine in natural layout.
"""

import numpy as np
import ml_dtypes

import concourse.bass as bass
import concourse.bacc as bacc
import concourse.mybir as mybir
import concourse.tile as tile
from concourse.bass_utils import run_bass_kernel_spmd

P = 128
D = 512
H = 8
DH = 64
B, Q, KLEN = 2, 2048, 2048
QS = 512
NCORES = 8
PB = NCORES // B
SCALE = 0.125
LN_EPS = 1e-5

F32 = mybir.dt.float32
BF16 = mybir.dt.bfloat16
F8 = mybir.dt.float8e4
AF = mybir.ActivationFunctionType
OP = mybir.AluOpType
DRM = mybir.MatmulPerfMode.DoubleRow

NPF8 = ml_dtypes.float8_e4m3
NPBF = ml_dtypes.bfloat16


def _build(NJC: int, QA: int) -> bass.Bass:
    KPC = NJC * P
    NQA = QA // P
    NPR = NJC // 2
    TAIL = NJC - 2 * NPR

    nc = bacc.Bacc("TRN2", target_bir_lowering=False, debug=False)

    din = {}
    for name, shape, dt in (
        ("q", [QS, D], F32),
        ("qt", [2, P, 2, D], F8),
        ("k", [KPC, D], BF16),
        ("v", [KPC, D], BF16),
        ("wq", [2, P, 2, D], F8),
        ("wk", [2, P, 2, D], F8),
        ("wv", [2, P, 2, D], F8),
        ("wo", [DH, H, D], F8),
        ("gw", [4, P, 2, D], F8),
        ("gb", [P, 4], F32),
        ("kones", [P, NJC], F8),
        ("qm", [DH, QS], F32),
    ):
        din[name] = nc.dram_tensor(name, shape, dt, kind="ExternalInput")
    out_d = nc.dram_tensor("out", [QS, D], F32, kind="ExternalOutput")

    with tile.TileContext(nc) as tc:
        _body(nc, tc, din, out_d, NJC, QA, KPC, NQA, NPR, TAIL)
    nc.compile()
    return nc


def _body(nc, tc, din, out_d, NJC, QA, KPC, NQA, NPR, TAIL):
    from contextlib import ExitStack

    ctx = ExitStack()
    with ctx:
        persist = ctx.enter_context(tc.tile_pool(name="persist", bufs=1))
        stage = ctx.enter_context(tc.tile_pool(name="stage", bufs=1))
        stats = ctx.enter_context(tc.tile_pool(name="stats", bufs=4))
        nbuf = ctx.enter_context(tc.tile_pool(name="nbuf", bufs=4))
        pexp = ctx.enter_context(tc.tile_pool(name="pexp", bufs=2))
        prec = ctx.enter_context(tc.tile_pool(name="prec", bufs=4))
        cmb = ctx.enter_context(tc.tile_pool(name="cmb", bufs=8))
        # PSUM: 2 + 2*2 + 2 = 8 banks
        pacc = ctx.enter_context(tc.tile_pool(name="pacc", bufs=2, space="PSUM"))
        pS = ctx.enter_context(tc.tile_pool(name="pS", bufs=2, space="PSUM"))
        pnd = ctx.enter_context(tc.tile_pool(name="pnd", bufs=1, space="PSUM"))

        # ---- persistent inputs ----
        eps_t = persist.tile([P, 1], F32)
        nc.vector.memset(eps_t, LN_EPS)
        negone_t = persist.tile([P, 1], F32)
        nc.vector.memset(negone_t, -1.0)
        wq_t = persist.tile([P, 2, 2, D], F8, name="wq_t")
        wk_t = persist.tile([P, 2, 2, D], F8, name="wk_t")
        wv_t = persist.tile([P, 2, 2, D], F8, name="wv_t")
        for wt, wn in ((wk_t, "wk"), (wq_t, "wq"), (wv_t, "wv")):
            nc.sync.dma_start(out=wt, in_=din[wn][...].rearrange("j p i d -> p j i d"))
        wo_t = persist.tile([DH, H, D], F8, name="wo_t")
        nc.scalar.dma_start(out=wo_t, in_=din["wo"][...])
        gw_t = persist.tile([P, 4, 2, D], F8, name="gw_t")
        nc.scalar.dma_start(out=gw_t, in_=din["gw"][...].rearrange("j p i d -> p j i d"))
        gb_t = persist.tile([P, 4], F32, name="gb_t")
        nc.sync.dma_start(out=gb_t, in_=din["gb"][...])
        qt_t = persist.tile([P, 2, 2, D], F8, name="qt_t")
        nc.scalar.dma_start(out=qt_t, in_=din["qt"][...].rearrange("j p i d -> p j i d"))
        kones_t = persist.tile([P, NJC], F8, name="kones_t")
        nc.sync.dma_start(out=kones_t, in_=din["kones"][...])
        qm_t = persist.tile([DH, QS], F32, name="qm_t")
        nc.sync.dma_start(out=qm_t, in_=din["qm"][...])
        q_nat = persist.tile([P, 4, D], F32, name="q_nat")
        nc.sync.dma_start(out=q_nat, in_=din["q"][...].rearrange("(a p) d -> p a d", p=P))

        kones_mat = persist.tile([P, NJC, DH], F8, name="kones_mat")
        nc.gpsimd.tensor_copy(
            kones_mat, kones_t[...].unsqueeze(2).broadcast_to((P, NJC, DH))
        )

        # ---- persistent activations ----
        qn_bf = persist.tile([P, 4, D], BF16, name="qn_bf")
        q_bf = persist.tile([P, 4, D], BF16, name="q_bf")
        qnT_bf = persist.tile([P, 4, NQA * P], BF16, name="qnT_bf")
        qnT_f8 = persist.tile([P, 4, NQA * P], F8, name="qnT_f8")
        qhT = persist.tile([P, 4, QA], F8, name="qhT")
        knT_bf = persist.tile([P, 4, KPC], BF16, name="knT_bf")
        knT_f8 = persist.tile([P, 4, KPC], F8, name="knT_f8")
        khT = persist.tile([P, 4, KPC], F8, name="khT")
        vnT_bf = persist.tile([P, 4, KPC], BF16, name="vnT_bf")
        vnT_f8 = persist.tile([P, 4, KPC], F8, name="vnT_f8")
        vh_st = persist.tile([P, NJC, H, DH], F8, name="vh_st")
        av_t = persist.tile([DH, H, QS], F8, name="av_t")
        poT_f8 = persist.tile([P, 4, D], F8, name="poT_f8")
        poT_bf = persist.tile([P, 4, D], BF16, name="poT_bf")
        gT_bf = persist.tile([P, 4, D], BF16, name="gT_bf")
        po_nat = persist.tile([P, 4, D], BF16, name="po_nat")
        g_nat = persist.tile([P, 4, D], BF16, name="g_nat")
        out_nat = persist.tile([P, 4, D], F32, name="out_nat")

        if QA < QS:
            nc.gpsimd.memset(poT_f8[:, :, :].rearrange("p a d -> p (a d)"), 0.0)
            nc.gpsimd.memset(poT_bf[:, :, :].rearrange("p a d -> p (a d)"), 0.0)

        def ln_batch(chunks, nblk, norm_eng, dst_bf):
            """chunks: list of (c0, cw, tile). Batched stats -> one sqrt ->
            norms into dst_bf(c, tile_slice)."""
            mvall = stats.tile([P, nblk, 2], F32, name="mvall", bufs=2)
            for c0, cw, xst in chunks:
                for cc in range(cw):
                    st = stats.tile([P, 6], F32, name="bnst", bufs=8)
                    nc.vector.bn_stats(out=st, in_=xst[:, cc, :])
                    nc.vector.bn_aggr(out=mvall[:, c0 + cc, :], in_=st)
            std = stats.tile([P, nblk], F32, name="stdall", bufs=2)
            nc.scalar.activation(
                out=std, in_=mvall[:, :, 1], func=AF.Sqrt, bias=eps_t
            )
            rstd = stats.tile([P, nblk], F32, name="rstdall", bufs=2)
            nc.vector.reciprocal_approx_fast(out=rstd, in_=std)
            nm2 = stats.tile([P, nblk], F32, name="nm2all", bufs=2)
            nc.vector.tensor_tensor(
                out=nm2, in0=mvall[:, :, 0], in1=rstd, op=OP.mult
            )
            nc.vector.tensor_scalar_mul(nm2, nm2, -1.0)
            for c0, cw, xst in chunks:
                for cc in range(cw):
                    c = c0 + cc
                    norm_eng.tensor_scalar(
                        out=dst_bf(c),
                        in0=xst[:, cc, :],
                        scalar1=nm2[:, c : c + 1],
                        scalar2=rstd[:, c : c + 1],
                        op0=OP.add,
                        op1=OP.mult,
                    )

        def ln_T_cast(src_dram, nT_bf, nT_f8, norm_eng, dma_eng):
            """k/v: DMA chunks -> batched LN -> bf16 -> DMA-T -> fp8 cast."""
            chunks = []
            for c0 in range(0, NJC, 2):
                cw = min(2, NJC - c0)
                xst = stage.tile([P, 2, D], BF16, name="xst", bufs=12)
                dma_eng.dma_start(
                    out=xst[:, :cw, :],
                    in_=src_dram[c0 * P : (c0 + cw) * P, :].rearrange(
                        "(c p) d -> p c d", p=P
                    ),
                )
                chunks.append((c0, cw, xst))
            xn_tiles = {}

            def dst_bf(c):
                t = nbuf.tile([P, D], BF16, name="xn", bufs=6)
                xn_tiles[c] = t
                return t

            ln_batch(chunks, NJC, norm_eng, dst_bf)
            for c in range(NJC):
                dma_eng.dma_start(
                    out=nT_bf[:, :, c * P : (c + 1) * P], in_=xn_tiles[c],
                    transpose=True,
                )
                if c % 2 == 1 or c == NJC - 1:
                    c0 = c - (c % 2)
                    nc.vector.tensor_copy(
                        nT_f8[:, :, c0 * P : (c + 1) * P],
                        nT_bf[:, :, c0 * P : (c + 1) * P],
                    )

        # ---- k path + k proj ----
        ln_T_cast(din["k"], knT_bf, knT_f8, nc.gpsimd, nc.sync)
        for a in range(4):
            for n0 in range(0, KPC, 512):
                nw = min(512, KPC - n0)
                pp = pacc.tile([P, D], F32, name="pacc_t")
                for j in range(2):
                    nc.tensor.matmul(
                        pp[:, :nw],
                        wk_t[:, j, :, a * P : (a + 1) * P],
                        knT_f8[:, 2 * j : 2 * j + 2, n0 : n0 + nw],
                        start=(j == 0),
                        stop=(j == 1),
                        perf_mode=DRM,
                    )
                nc.vector.tensor_copy(khT[:, a, n0 : n0 + nw], pp[:, :nw])

        # ---- q: LN -> bf16 -> DMA-transpose (active blocks) -> fp8 ----
        qchunks = [(0, 2, q_nat[:, 0:2, :]), (2, 2, q_nat[:, 2:4, :])]
        ln_batch(qchunks, 4, nc.gpsimd, lambda c: qn_bf[:, c, :])
        for a in range(NQA):
            nc.scalar.dma_start(
                out=qnT_bf[:, :, a * P : (a + 1) * P], in_=qn_bf[:, a, :],
                transpose=True,
            )
        nc.vector.tensor_copy(qnT_f8, qnT_bf)
        nc.gpsimd.tensor_copy(q_bf, q_nat)

        # ---- q proj (DoubleRow) ----
        for a in range(4):
            pp = pacc.tile([P, D], F32, name="pacc_t")
            for j in range(2):
                nc.tensor.matmul(
                    pp[:, 0:QA],
                    wq_t[:, j, :, a * P : (a + 1) * P],
                    qnT_f8[:, 2 * j : 2 * j + 2, 0:QA],
                    start=(j == 0),
                    stop=(j == 1),
                    perf_mode=DRM,
                )
            nc.vector.tensor_copy(qhT[:, a, :], pp[:, 0:QA])

        # ---- v path + v proj into vh_st ----
        ln_T_cast(din["v"], vnT_bf, vnT_f8, nc.gpsimd, nc.scalar)
        for c in range(NJC):
            pp = pacc.tile([P, D], F32, name="pacc_t")
            for j in range(2):
                nc.tensor.matmul(
                    pp,
                    vnT_f8[:, 2 * j : 2 * j + 2, c * P : (c + 1) * P],
                    wv_t[:, j, :, :],
                    start=(j == 0),
                    stop=(j == 1),
                    perf_mode=DRM,
                )
            nc.scalar.copy(
                vh_st[:, c, :, :], pp[...].rearrange("p (h e) -> p h e", h=H)
            )

        # ---- attention, head pairs interleaved on PE row-tiles ----
        for hp in range(H // 2):
            expS = pexp.tile([P, NJC, 2, QA], F8, name="expS")
            for c in range(NJC):
                ps = pS.tile([P, 2, 512], F32, name="pS_t")
                for hh in range(2):
                    r0 = hh * DH
                    nc.tensor.matmul(
                        ps[:, hh, 0:QA],
                        khT[r0 : r0 + DH, hp, c * P : (c + 1) * P],
                        qhT[r0 : r0 + DH, hp, :],
                        start=True,
                        stop=True,
                    )
                nc.scalar.activation(
                    out=expS[:, c, :, :],
                    in_=ps[:, 0:2, 0:QA],
                    func=AF.Exp,
                    scale=SCALE,
                    bias=negone_t,
                )
            for hh in range(2):
                h = 2 * hp + hh
                pnum = pnd.tile([DH, 512], F32, name="pnum")
                pden = pnd.tile([DH, 512], F32, name="pden")
                for pr in range(NPR):
                    fl = dict(start=(pr == 0), stop=(TAIL == 0 and pr == NPR - 1))
                    nc.tensor.matmul(
                        pnum[:, 0:QA],
                        vh_st[:, 2 * pr : 2 * pr + 2, h, :],
                        expS[:, 2 * pr : 2 * pr + 2, hh, :],
                        perf_mode=DRM,
                        **fl,
                    )
                    nc.tensor.matmul(
                        pden[:, 0:QA],
                        kones_mat[:, 2 * pr : 2 * pr + 2, :],
                        expS[:, 2 * pr : 2 * pr + 2, hh, :],
                        perf_mode=DRM,
                        **fl,
                    )
                if TAIL:
                    nc.tensor.matmul(
                        pnum[:, 0:QA], vh_st[:, NJC - 1, h, :],
                        expS[:, NJC - 1, hh, :], start=(NPR == 0), stop=True,
                    )
                    nc.tensor.matmul(
                        pden[:, 0:QA], kones_mat[:, NJC - 1, :],
                        expS[:, NJC - 1, hh, :], start=(NPR == 0), stop=True,
                    )
                rec = prec.tile([DH, QA], F32, name="rec")
                nc.vector.reciprocal_approx_fast(out=rec, in_=pden[:, 0:QA])
                rec2 = prec.tile([DH, QA], F32, name="rec2")
                nc.vector.tensor_tensor(
                    out=rec2, in0=rec, in1=qm_t[:, 0:QA], op=OP.mult
                )
                nc.vector.tensor_tensor(
                    out=av_t[:, h, 0:QA], in0=pnum[:, 0:QA], in1=rec2, op=OP.mult
                )

        # ---- output projection (plain fp8, contraction 64 per head) ----
        for a in range(4):
            pp = pacc.tile([P, D], F32, name="pacc_t")
            for h in range(H):
                nc.tensor.matmul(
                    pp[:, 0:QA],
                    wo_t[:, h, a * P : (a + 1) * P],
                    av_t[:, h, 0:QA],
                    start=(h == 0),
                    stop=(h == H - 1),
                )
            nc.scalar.copy(poT_f8[:, a, 0:QA], pp[:, 0:QA])
            nc.vector.tensor_copy(poT_bf[:, a, 0:QA], pp[:, 0:QA])

        # ---- gate (DoubleRow over [q; po], K=1024) ----
        for a in range(4):
            pp = pacc.tile([P, D], F32, name="pacc_t")
            for j in range(4):
                rhs = (
                    qt_t[:, j, :, :]
                    if j < 2
                    else poT_f8[:, 2 * (j - 2) : 2 * (j - 2) + 2, :]
                )
                nc.tensor.matmul(
                    pp,
                    gw_t[:, j, :, a * P : (a + 1) * P],
                    rhs,
                    start=(j == 0),
                    stop=(j == 3),
                    perf_mode=DRM,
                )
            nc.scalar.activation(
                out=gT_bf[:, a, :], in_=pp, func=AF.Sigmoid, bias=gb_t[:, a : a + 1]
            )

        # ---- back to natural layout + combine (bf16, 2x DVE mode) ----
        for a in range(4):
            nc.scalar.dma_start(
                out=po_nat[:, :, a * P : (a + 1) * P], in_=poT_bf[:, a, :],
                transpose=True,
            )
            nc.sync.dma_start(
                out=g_nat[:, :, a * P : (a + 1) * P], in_=gT_bf[:, a, :],
                transpose=True,
            )
        for a in range(4):
            s = cmb.tile([P, D], BF16, name="cmb_t")
            nc.vector.tensor_tensor(
                out=s, in0=q_bf[:, a, :], in1=po_nat[:, a, :], op=OP.subtract
            )
            r = cmb.tile([P, D], BF16, name="cmb_t")
            nc.gpsimd.tensor_tensor(
                out=r, in0=q_bf[:, a, :], in1=po_nat[:, a, :], op=OP.add
            )
            m = cmb.tile([P, D], BF16, name="cmb_t")
            nc.vector.tensor_tensor(out=m, in0=g_nat[:, a, :], in1=s, op=OP.mult)
            nc.vector.tensor_tensor(out=out_nat[:, a, :], in0=m, in1=r, op=OP.add)

        nc.sync.dma_start(
            out=out_d[:, :].rearrange("(a p) d -> p a d", p=P), in_=out_nat
        )


_CACHE: dict = {}


def make_in_maps(inputs):
    q = np.asarray(inputs["query"], np.float32)
    k = np.asarray(inputs["key"], np.float32)
    v = np.asarray(inputs["value"], np.float32)
    wq = np.asarray(inputs["weight_q"], np.float32)
    wk = np.asarray(inputs["weight_k"], np.float32)
    wv = np.asarray(inputs["weight_v"], np.float32)
    wo = np.asarray(inputs["weight_o"], np.float32)
    gw = np.asarray(inputs["g_w"], np.float32)
    gb = np.asarray(inputs["g_b"], np.float32)
    qmask = np.asarray(inputs["query_mask"])
    kmask = np.asarray(inputs["key_mask"])
    gams = {n: np.asarray(inputs[n], np.float32) for n in ("q_gamma", "k_gamma", "v_gamma")}
    bets = [np.asarray(inputs[n], np.float32) for n in ("q_beta", "k_beta", "v_beta")]
    if any(np.any(bt != 0.0) for bt in bets):
        raise NotImplementedError("nonzero LN beta not supported")

    # gamma folds into the projection weights: (z*g) @ W == z @ (diag(g) W)
    wq = gams["q_gamma"][:, None] * wq
    wk = gams["k_gamma"][:, None] * wk
    wv = gams["v_gamma"][:, None] * wv

    def dr4(w):  # [D, D] -> [2, 128, 2, D] DoubleRow-interleaved, fp8
        return np.ascontiguousarray(
            w.reshape(2, 2, P, D).transpose(0, 2, 1, 3)
        ).astype(NPF8)

    wq8, wk8, wv8 = dr4(wq), dr4(wk), dr4(wv)
    wo8 = np.ascontiguousarray(wo.reshape(H, DH, D).transpose(1, 0, 2)).astype(NPF8)
    gw8 = np.ascontiguousarray(
        gw.reshape(4, 2, P, D).transpose(0, 2, 1, 3)
    ).astype(NPF8)
    gb_cm = np.ascontiguousarray(gb.reshape(4, P).T)

    # key compaction: keep mask!=0, append zero-attn slot, pad to NJC*128
    kept = [np.nonzero(kmask[b])[0] for b in range(B)]
    nkp = [len(ix) + 1 for ix in kept]
    NJC = max(1, (max(nkp) + P - 1) // P)
    KPC = NJC * P
    k_in = np.zeros((B, KPC, D), NPBF)
    v_in = np.zeros((B, KPC, D), NPBF)
    kones = np.zeros((B, P, NJC), NPF8)
    for b in range(B):
        k_in[b, : nkp[b] - 1] = k[b, kept[b]].astype(NPBF)
        v_in[b, : nkp[b] - 1] = v[b, kept[b]].astype(NPBF)
        ar = np.zeros(KPC, np.float32)
        ar[: nkp[b]] = 1.0
        kones[b] = ar.reshape(NJC, P).T.astype(NPF8)

    # query rows: active-first permutation per core
    rows = []
    for b in range(B):
        act = np.nonzero(qmask[b])[0]
        inact = np.nonzero(qmask[b] == 0)[0]
        acts = [act[r::PB] for r in range(PB)]
        pos = 0
        for r in range(PB):
            need = QS - len(acts[r])
            rows.append((b, np.concatenate([acts[r], inact[pos : pos + need]])))
            pos += need
        assert pos == len(inact)
    max_act = max(int(np.sum(qmask[b][r] != 0)) for b, r in rows)
    QA = min(QS, max(P, ((max_act + P - 1) // P) * P))

    in_maps = []
    for c in range(NCORES):
        b, rw = rows[c]
        qc = np.ascontiguousarray(q[b, rw])
        qt8 = np.ascontiguousarray(
            qc.T.reshape(2, 2, P, QS).transpose(0, 2, 1, 3)
        ).astype(NPF8)
        qm_bc = np.broadcast_to(
            (qmask[b, rw] != 0).astype(np.float32)[None, :], (DH, QS)
        )
        in_maps.append(
            {
                "q": qc,
                "qt": qt8,
                "k": k_in[b],
                "v": v_in[b],
                "wq": wq8,
                "wk": wk8,
                "wv": wv8,
                "wo": wo8,
                "gw": gw8,
                "gb": gb_cm,
                "kones": kones[b],
                "qm": np.ascontiguousarray(qm_bc),
            }
        )
    return in_maps, rows, (NJC, QA)


def kernel(_return_res=False, _run_kwargs=None, **inputs):
    run_kwargs = _run_kwargs or {}
    in_maps, rows, key = make_in_maps(inputs)
    if key not in _CACHE:
        _CACHE[key] = _build(*key)
    nc = _CACHE[key]
    res = run_bass_kernel_spmd(nc, in_maps, list(range(NCORES)), **run_kwargs)
    out = np.empty((B, Q, D), np.float32)
    for c in range(NCORES):
        b, rw = rows[c]
        out[b, rw] = res.results[c]["out"]
    if _return_res:
        return out, res
    return out


# revision 11
# speedup vs baseline: 2.1478x; 1.1300x over previous
"""Trainium2 Bass kernel for BaseAttnPredictNet (pre-LN MHA with zero-attn
slot, gated output combination, residual).

Sharding: data-parallel over (batch, query-rows); 8 cores, 512 q rows each.

Host-side prep (layout only, no math): keys with mask==0 are dropped per
batch (attention is permutation-invariant over keys) and a zero-attn slot
appended; query rows are permuted active-first per core so attention runs
only on the first QA columns; weights are cast fp8 and pre-interleaved for
DoubleRow matmuls; the gate's query operand is pre-transposed.

On-device: LN in natural layout (batched DVE stats, Pool normalize),
transposes via the HWDGE DMA crossbar (no PE transposes), fp8 DoubleRow
projections, plain-fp8 64-contraction scores interleaved across head pairs
on opposite PE row-tiles, softmax without max-subtraction (exp(s/8 - 1),
fp8 out), PV as per-head DoubleRow matmuls producing transposed attention
output plus a ones-matmul for denominators (pad keys excluded via a 0/1
stationary), division folded with the query mask, plain-fp8 output
projection, DoubleRow gate, bf16 combine in natural layout.
"""

import numpy as np
import ml_dtypes

import concourse.bass as bass
import concourse.bacc as bacc
import concourse.mybir as mybir
import concourse.tile as tile
from concourse.bass_utils import run_bass_kernel_spmd

P = 128
D = 512
H = 8
DH = 64
B, Q, KLEN = 2, 2048, 2048
QS = 512
NCORES = 8
PB = NCORES // B
SCALE = 0.125
LN_EPS = 1e-5

F32 = mybir.dt.float32
BF16 = mybir.dt.bfloat16
F8 = mybir.dt.float8e4
AF = mybir.ActivationFunctionType
OP = mybir.AluOpType
DRM = mybir.MatmulPerfMode.DoubleRow

NPF8 = ml_dtypes.float8_e4m3
NPBF = ml_dtypes.bfloat16


def _build(NJC: int, QA: int) -> bass.Bass:
    KPC = NJC * P
    NQA = (QA + P - 1) // P
    NPR = NJC // 2
    TAIL = NJC - 2 * NPR

    nc = bacc.Bacc("TRN2", target_bir_lowering=False, debug=False)

    din = {}
    for name, shape, dt in (
        ("q", [QS, D], F32),
        ("qt", [2, P, 2, D], F8),
        ("k", [KPC, D], BF16),
        ("v", [KPC, D], BF16),
        ("wq", [2, P, 2, D], F8),
        ("wk", [2, P, 2, D], F8),
        ("wv", [2, P, 2, D], F8),
        ("wo", [DH, H, D], F8),
        ("gw", [4, P, 2, D], F8),
        ("gb", [P, 4], F32),
        ("kones", [P, NJC], F8),
        ("qm", [DH, QS], F32),
    ):
        din[name] = nc.dram_tensor(name, shape, dt, kind="ExternalInput")
    out_d = nc.dram_tensor("out", [QS, D], F32, kind="ExternalOutput")

    with tile.TileContext(nc) as tc:
        _body(nc, tc, din, out_d, NJC, QA, KPC, NQA, NPR, TAIL)
    nc.compile()
    return nc


def _body(nc, tc, din, out_d, NJC, QA, KPC, NQA, NPR, TAIL):
    from contextlib import ExitStack

    ctx = ExitStack()
    with ctx:
        persist = ctx.enter_context(tc.tile_pool(name="persist", bufs=1))
        stage = ctx.enter_context(tc.tile_pool(name="stage", bufs=1))
        stats = ctx.enter_context(tc.tile_pool(name="stats", bufs=4))
        nbuf = ctx.enter_context(tc.tile_pool(name="nbuf", bufs=4))
        pexp = ctx.enter_context(tc.tile_pool(name="pexp", bufs=2))
        prec = ctx.enter_context(tc.tile_pool(name="prec", bufs=4))
        cmb = ctx.enter_context(tc.tile_pool(name="cmb", bufs=8))
        # PSUM: 2 + 2*2 + 2 = 8 banks
        pacc = ctx.enter_context(tc.tile_pool(name="pacc", bufs=2, space="PSUM"))
        pS = ctx.enter_context(tc.tile_pool(name="pS", bufs=2, space="PSUM"))
        pnd = ctx.enter_context(tc.tile_pool(name="pnd", bufs=1, space="PSUM"))

        # ---- persistent inputs ----
        eps_t = persist.tile([P, 1], F32)
        nc.vector.memset(eps_t, LN_EPS)
        negone_t = persist.tile([P, 1], F32)
        nc.vector.memset(negone_t, -1.0)
        wq_t = persist.tile([P, 2, 2, D], F8, name="wq_t")
        wk_t = persist.tile([P, 2, 2, D], F8, name="wk_t")
        wv_t = persist.tile([P, 2, 2, D], F8, name="wv_t")
        for wt, wn in ((wk_t, "wk"), (wq_t, "wq"), (wv_t, "wv")):
            nc.sync.dma_start(out=wt, in_=din[wn][...].rearrange("j p i d -> p j i d"))
        wo_t = persist.tile([DH, H, D], F8, name="wo_t")
        nc.scalar.dma_start(out=wo_t, in_=din["wo"][...])
        gw_t = persist.tile([P, 4, 2, D], F8, name="gw_t")
        nc.scalar.dma_start(out=gw_t, in_=din["gw"][...].rearrange("j p i d -> p j i d"))
        gb_t = persist.tile([P, 4], F32, name="gb_t")
        nc.sync.dma_start(out=gb_t, in_=din["gb"][...])
        qt_t = persist.tile([P, 2, 2, D], F8, name="qt_t")
        nc.scalar.dma_start(out=qt_t, in_=din["qt"][...].rearrange("j p i d -> p j i d"))
        kones_t = persist.tile([P, NJC], F8, name="kones_t")
        nc.sync.dma_start(out=kones_t, in_=din["kones"][...])
        qm_t = persist.tile([DH, QS], F32, name="qm_t")
        nc.sync.dma_start(out=qm_t, in_=din["qm"][...])
        q_nat = persist.tile([P, 4, D], F32, name="q_nat")
        nc.sync.dma_start(out=q_nat, in_=din["q"][...].rearrange("(a p) d -> p a d", p=P))

        # PE p-state warmup: dummy matmuls keep the clock ramping while
        # the LN prologue runs; they have no consumers.
        warm = persist.tile([P, 128], F8, name="warm")
        nc.gpsimd.memset(warm, 0.25)
        pwarm = pacc.tile([P, 512], F32, name="pacc_t")
        for _ in range(40):
            nc.tensor.matmul(
                pwarm, warm, wk_t[:, 0, 0, :], start=True, stop=True,
                skip_group_check=True,
            )

        kones_mat = persist.tile([P, NJC, DH], F8, name="kones_mat")
        nc.gpsimd.tensor_copy(
            kones_mat, kones_t[...].unsqueeze(2).broadcast_to((P, NJC, DH))
        )

        # ---- persistent activations ----
        qn_bf = persist.tile([P, 4, D], BF16, name="qn_bf")
        q_bf = persist.tile([P, 4, D], BF16, name="q_bf")
        qnT_bf = persist.tile([P, 4, NQA * P], BF16, name="qnT_bf")
        qnT_f8 = persist.tile([P, 4, NQA * P], F8, name="qnT_f8")
        qhT = persist.tile([P, 4, QA], F8, name="qhT")
        knT_bf = persist.tile([P, 4, KPC], BF16, name="knT_bf")
        knT_f8 = persist.tile([P, 4, KPC], F8, name="knT_f8")
        khT = persist.tile([P, 4, KPC], F8, name="khT")
        vnT_bf = persist.tile([P, 4, KPC], BF16, name="vnT_bf")
        vnT_f8 = persist.tile([P, 4, KPC], F8, name="vnT_f8")
        vh_st = persist.tile([P, NJC, H, DH], F8, name="vh_st")
        av_t = persist.tile([DH, H, QS], F8, name="av_t")
        poT_f8 = persist.tile([P, 4, D], F8, name="poT_f8")
        poT_bf = persist.tile([P, 4, D], BF16, name="poT_bf")
        gT_bf = persist.tile([P, 4, D], BF16, name="gT_bf")
        po_nat = persist.tile([P, 4, D], BF16, name="po_nat")
        g_nat = persist.tile([P, 4, D], BF16, name="g_nat")
        out_nat = persist.tile([P, 4, D], F32, name="out_nat")

        def ln_batch(chunks, nblk, norm_eng, dst_bf):
            """chunks: list of (c0, cw, tile). Batched stats -> one sqrt ->
            norms into dst_bf(c, tile_slice)."""
            mvall = stats.tile([P, nblk, 2], F32, name="mvall", bufs=2)
            for c0, cw, xst in chunks:
                for cc in range(cw):
                    st = stats.tile([P, 6], F32, name="bnst", bufs=8)
                    nc.vector.bn_stats(out=st, in_=xst[:, cc, :])
                    nc.vector.bn_aggr(out=mvall[:, c0 + cc, :], in_=st)
            std = stats.tile([P, nblk], F32, name="stdall", bufs=2)
            nc.scalar.activation(
                out=std, in_=mvall[:, :, 1], func=AF.Sqrt, bias=eps_t
            )
            rstd = stats.tile([P, nblk], F32, name="rstdall", bufs=2)
            nc.vector.reciprocal_approx_fast(out=rstd, in_=std)
            nm2 = stats.tile([P, nblk], F32, name="nm2all", bufs=2)
            nc.vector.tensor_tensor(
                out=nm2, in0=mvall[:, :, 0], in1=rstd, op=OP.mult
            )
            nc.vector.tensor_scalar_mul(nm2, nm2, -1.0)
            for c0, cw, xst in chunks:
                for cc in range(cw):
                    c = c0 + cc
                    norm_eng.tensor_scalar(
                        out=dst_bf(c),
                        in0=xst[:, cc, :],
                        scalar1=nm2[:, c : c + 1],
                        scalar2=rstd[:, c : c + 1],
                        op0=OP.add,
                        op1=OP.mult,
                    )

        def ln_T_cast(src_dram, nT_bf, nT_f8, norm_eng, dma_eng, t_eng=None):
            """k/v: DMA chunks -> batched LN -> bf16 -> DMA-T -> fp8 cast."""
            chunks = []
            for c0 in range(0, NJC, 2):
                cw = min(2, NJC - c0)
                xst = stage.tile([P, 2, D], BF16, name="xst", bufs=12)
                dma_eng.dma_start(
                    out=xst[:, :cw, :],
                    in_=src_dram[c0 * P : (c0 + cw) * P, :].rearrange(
                        "(c p) d -> p c d", p=P
                    ),
                )
                chunks.append((c0, cw, xst))
            xn_tiles = {}

            def dst_bf(c):
                t = nbuf.tile([P, D], BF16, name="xn", bufs=6)
                xn_tiles[c] = t
                return t

            ln_batch(chunks, NJC, norm_eng, dst_bf)
            for c in range(NJC):
                te = dma_eng if (t_eng is None or c % 2 == 0) else t_eng
                te.dma_start(
                    out=nT_bf[:, :, c * P : (c + 1) * P], in_=xn_tiles[c],
                    transpose=True,
                )
                if c % 2 == 1 or c == NJC - 1:
                    c0 = c - (c % 2)
                    nc.vector.tensor_copy(
                        nT_f8[:, :, c0 * P : (c + 1) * P],
                        nT_bf[:, :, c0 * P : (c + 1) * P],
                    )

        # ---- k path + k proj ----
        ln_T_cast(din["k"], knT_bf, knT_f8, nc.gpsimd, nc.sync, t_eng=nc.scalar)
        for a in range(4):
            for n0 in range(0, KPC, 512):
                nw = min(512, KPC - n0)
                pp = pacc.tile([P, D], F32, name="pacc_t")
                for j in range(2):
                    nc.tensor.matmul(
                        pp[:, :nw],
                        wk_t[:, j, :, a * P : (a + 1) * P],
                        knT_f8[:, 2 * j : 2 * j + 2, n0 : n0 + nw],
                        start=(j == 0),
                        stop=(j == 1),
                        perf_mode=DRM,
                    )
                nc.vector.tensor_copy(khT[:, a, n0 : n0 + nw], pp[:, :nw])

        # ---- q: LN -> bf16 -> DMA-transpose (active blocks) -> fp8 ----
        qchunks = [(0, 2, q_nat[:, 0:2, :]), (2, 2, q_nat[:, 2:4, :])]
        ln_batch(qchunks, 4, nc.gpsimd, lambda c: qn_bf[:, c, :])
        for a in range(NQA):
            nc.sync.dma_start(
                out=qnT_bf[:, :, a * P : (a + 1) * P], in_=qn_bf[:, a, :],
                transpose=True,
            )
        nc.vector.tensor_copy(qnT_f8, qnT_bf)

        # ---- q proj (DoubleRow) ----
        for a in range(4):
            pp = pacc.tile([P, D], F32, name="pacc_t")
            for j in range(2):
                nc.tensor.matmul(
                    pp[:, 0:QA],
                    wq_t[:, j, :, a * P : (a + 1) * P],
                    qnT_f8[:, 2 * j : 2 * j + 2, 0:QA],
                    start=(j == 0),
                    stop=(j == 1),
                    perf_mode=DRM,
                )
            nc.vector.tensor_copy(qhT[:, a, :], pp[:, 0:QA])

        # ---- v path + v proj into vh_st ----
        ln_T_cast(din["v"], vnT_bf, vnT_f8, nc.gpsimd, nc.scalar, t_eng=nc.sync)
        for c in range(NJC):
            pp = pacc.tile([P, D], F32, name="pacc_t")
            for j in range(2):
                nc.tensor.matmul(
                    pp,
                    vnT_f8[:, 2 * j : 2 * j + 2, c * P : (c + 1) * P],
                    wv_t[:, j, :, :],
                    start=(j == 0),
                    stop=(j == 1),
                    perf_mode=DRM,
                )
            nc.vector.tensor_copy(
                vh_st[:, c, :, :], pp[...].rearrange("p (h e) -> p h e", h=H)
            )

        # ---- attention, head pairs interleaved on PE row-tiles ----
        for hp in range(H // 2):
            expS = pexp.tile([P, NJC, 2, QA], F8, name="expS")
            for c in range(NJC):
                ps = pS.tile([P, 2, 512], F32, name="pS_t")
                for hh in range(2):
                    r0 = hh * DH
                    nc.tensor.matmul(
                        ps[:, hh, 0:QA],
                        khT[r0 : r0 + DH, hp, c * P : (c + 1) * P],
                        qhT[r0 : r0 + DH, hp, :],
                        start=True,
                        stop=True,
                    )
                nc.scalar.activation(
                    out=expS[:, c, :, :],
                    in_=ps[:, 0:2, 0:QA],
                    func=AF.Exp,
                    scale=SCALE,
                    bias=negone_t,
                )
            for hh in range(2):
                h = 2 * hp + hh
                pnum = pnd.tile([DH, 512], F32, name="pnum")
                pden = pnd.tile([DH, 512], F32, name="pden")
                for pr in range(NPR):
                    fl = dict(start=(pr == 0), stop=(TAIL == 0 and pr == NPR - 1))
                    nc.tensor.matmul(
                        pnum[:, 0:QA],
                        vh_st[:, 2 * pr : 2 * pr + 2, h, :],
                        expS[:, 2 * pr : 2 * pr + 2, hh, :],
                        perf_mode=DRM,
                        **fl,
                    )
                    nc.tensor.matmul(
                        pden[:, 0:QA],
                        kones_mat[:, 2 * pr : 2 * pr + 2, :],
                        expS[:, 2 * pr : 2 * pr + 2, hh, :],
                        perf_mode=DRM,
                        **fl,
                    )
                if TAIL:
                    nc.tensor.matmul(
                        pnum[:, 0:QA], vh_st[:, NJC - 1, h, :],
                        expS[:, NJC - 1, hh, :], start=(NPR == 0), stop=True,
                    )
                    nc.tensor.matmul(
                        pden[:, 0:QA], kones_mat[:, NJC - 1, :],
                        expS[:, NJC - 1, hh, :], start=(NPR == 0), stop=True,
                    )
                rec = prec.tile([DH, QA], F32, name="rec")
                nc.vector.reciprocal_approx_fast(out=rec, in_=pden[:, 0:QA])
                rec2 = prec.tile([DH, QA], F32, name="rec2")
                nc.vector.tensor_tensor(
                    out=rec2, in0=rec, in1=qm_t[:, 0:QA], op=OP.mult
                )
                nc.vector.tensor_tensor(
                    out=av_t[:, h, 0:QA], in0=pnum[:, 0:QA], in1=rec2, op=OP.mult
                )

        for a in range(4):
            nc.vector.tensor_copy(q_bf[:, a, :], q_nat[:, a, :])
        if QA < QS:
            nc.gpsimd.memset(poT_f8[:, :, QA:], 0.0)
            nc.gpsimd.memset(poT_bf[:, :, QA:], 0.0)

        # ---- output projection (plain fp8, contraction 64 per head) ----
        for a in range(4):
            pp = pacc.tile([P, D], F32, name="pacc_t")
            for h in range(H):
                nc.tensor.matmul(
                    pp[:, 0:QA],
                    wo_t[:, h, a * P : (a + 1) * P],
                    av_t[:, h, 0:QA],
                    start=(h == 0),
                    stop=(h == H - 1),
                )
            nc.scalar.copy(poT_f8[:, a, 0:QA], pp[:, 0:QA])
            nc.vector.tensor_copy(poT_bf[:, a, 0:QA], pp[:, 0:QA])

        # ---- gate (DoubleRow over [q; po], K=1024) ----
        for a in range(4):
            pp = pacc.tile([P, D], F32, name="pacc_t")
            for j in range(4):
                rhs = (
                    qt_t[:, j, :, :]
                    if j < 2
                    else poT_f8[:, 2 * (j - 2) : 2 * (j - 2) + 2, :]
                )
                nc.tensor.matmul(
                    pp,
                    gw_t[:, j, :, a * P : (a + 1) * P],
                    rhs,
                    start=(j == 0),
                    stop=(j == 3),
                    perf_mode=DRM,
                )
            nc.scalar.activation(
                out=gT_bf[:, a, :], in_=pp, func=AF.Sigmoid, bias=gb_t[:, a : a + 1]
            )

        # ---- back to natural layout + combine (bf16, 2x DVE mode) ----
        for a in range(4):
            nc.scalar.dma_start(
                out=po_nat[:, :, a * P : (a + 1) * P], in_=poT_bf[:, a, :],
                transpose=True,
            )
            nc.sync.dma_start(
                out=g_nat[:, :, a * P : (a + 1) * P], in_=gT_bf[:, a, :],
                transpose=True,
            )
        for a in range(4):
            s = cmb.tile([P, D], BF16, name="cmb_t")
            nc.vector.tensor_tensor(
                out=s, in0=q_bf[:, a, :], in1=po_nat[:, a, :], op=OP.subtract
            )
            r = cmb.tile([P, D], BF16, name="cmb_t")
            nc.gpsimd.tensor_tensor(
                out=r, in0=q_bf[:, a, :], in1=po_nat[:, a, :], op=OP.add
            )
            m = cmb.tile([P, D], BF16, name="cmb_t")
            nc.vector.tensor_tensor(out=m, in0=g_nat[:, a, :], in1=s, op=OP.mult)
            nc.vector.tensor_tensor(out=out_nat[:, a, :], in0=m, in1=r, op=OP.add)

        nc.sync.dma_start(
            out=out_d[:, :].rearrange("(a p) d -> p a d", p=P), in_=out_nat
        )


_CACHE: dict = {}


def make_in_maps(inputs):
    q = np.asarray(inputs["query"], np.float32)
    k = np.asarray(inputs["key"], np.float32)
    v = np.asarray(inputs["value"], np.float32)
    wq = np.asarray(inputs["weight_q"], np.float32)
    wk = np.asarray(inputs["weight_k"], np.float32)
    wv = np.asarray(inputs["weight_v"], np.float32)
    wo = np.asarray(inputs["weight_o"], np.float32)
    gw = np.asarray(inputs["g_w"], np.float32)
    gb = np.asarray(inputs["g_b"], np.float32)
    qmask = np.asarray(inputs["query_mask"])
    kmask = np.asarray(inputs["key_mask"])
    gams = {n: np.asarray(inputs[n], np.float32) for n in ("q_gamma", "k_gamma", "v_gamma")}
    bets = [np.asarray(inputs[n], np.float32) for n in ("q_beta", "k_beta", "v_beta")]
    if any(np.any(bt != 0.0) for bt in bets):
        raise NotImplementedError("nonzero LN beta not supported")

    # gamma folds into the projection weights: (z*g) @ W == z @ (diag(g) W)
    wq = gams["q_gamma"][:, None] * wq
    wk = gams["k_gamma"][:, None] * wk
    wv = gams["v_gamma"][:, None] * wv

    def dr4(w):  # [D, D] -> [2, 128, 2, D] DoubleRow-interleaved, fp8
        return np.ascontiguousarray(
            w.reshape(2, 2, P, D).transpose(0, 2, 1, 3)
        ).astype(NPF8)

    wq8, wk8, wv8 = dr4(wq), dr4(wk), dr4(wv)
    wo8 = np.ascontiguousarray(wo.reshape(H, DH, D).transpose(1, 0, 2)).astype(NPF8)
    gw8 = np.ascontiguousarray(
        gw.reshape(4, 2, P, D).transpose(0, 2, 1, 3)
    ).astype(NPF8)
    gb_cm = np.ascontiguousarray(gb.reshape(4, P).T)

    # key compaction: keep mask!=0, append zero-attn slot, pad to NJC*128
    kept = [np.nonzero(kmask[b])[0] for b in range(B)]
    nkp = [len(ix) + 1 for ix in kept]
    NJC = max(1, (max(nkp) + P - 1) // P)
    KPC = NJC * P
    k_in = np.zeros((B, KPC, D), NPBF)
    v_in = np.zeros((B, KPC, D), NPBF)
    kones = np.zeros((B, P, NJC), NPF8)
    for b in range(B):
        k_in[b, : nkp[b] - 1] = k[b, kept[b]].astype(NPBF)
        v_in[b, : nkp[b] - 1] = v[b, kept[b]].astype(NPBF)
        ar = np.zeros(KPC, np.float32)
        ar[: nkp[b]] = 1.0
        kones[b] = ar.reshape(NJC, P).T.astype(NPF8)

    # query rows: active-first permutation per core
    rows = []
    for b in range(B):
        act = np.nonzero(qmask[b])[0]
        inact = np.nonzero(qmask[b] == 0)[0]
        acts = [act[r::PB] for r in range(PB)]
        pos = 0
        for r in range(PB):
            need = QS - len(acts[r])
            rows.append((b, np.concatenate([acts[r], inact[pos : pos + need]])))
            pos += need
        assert pos == len(inact)
    max_act = max(int(np.sum(qmask[b][r] != 0)) for b, r in rows)
    QA = min(QS, max(P, ((max_act + 63) // 64) * 64))

    in_maps = []
    for c in range(NCORES):
        b, rw = rows[c]
        qc = np.ascontiguousarray(q[b, rw])
        qt8 = np.ascontiguousarray(
            qc.T.reshape(2, 2, P, QS).transpose(0, 2, 1, 3)
        ).astype(NPF8)
        qm_bc = np.broadcast_to(
            (qmask[b, rw] != 0).astype(np.float32)[None, :], (DH, QS)
        )
        in_maps.append(
            {
                "q": qc,
                "qt": qt8,
                "k": k_in[b],
                "v": v_in[b],
                "wq": wq8,
                "wk": wk8,
                "wv": wv8,
                "wo": wo8,
                "gw": gw8,
                "gb": gb_cm,
                "kones": kones[b],
                "qm": np.ascontiguousarray(qm_bc),
            }
        )
    return in_maps, rows, (NJC, QA)


def kernel(_return_res=False, _run_kwargs=None, **inputs):
    run_kwargs = _run_kwargs or {}
    in_maps, rows, key = make_in_maps(inputs)
    if key not in _CACHE:
        _CACHE[key] = _build(*key)
    nc = _CACHE[key]
    res = run_bass_kernel_spmd(nc, in_maps, list(range(NCORES)), **run_kwargs)
    out = np.empty((B, Q, D), np.float32)
    for c in range(NCORES):
        b, rw = rows[c]
        out[b, rw] = res.results[c]["out"]
    if _return_res:
        return out, res
    return out
